# revision 2
# baseline (speedup 1.0000x reference)
"""Bidirectional 2-layer LSTM (with replicated hf1-input bug) + per-step linear,
as a Trainium2 Bass/Tile kernel, data-parallel over batch across 8 NeuronCores.

Layout strategy (per core, B_loc=256 batch):
  - packed state tile pkq [128 rows, 4 slots, 256 batch] (slot = wave%4):
      row 0 = x[t], row 1 = xb[t], row 2 = ones, rows 3:32 zero pad,
      rows 32:64 = hf0, rows 64:96 = hf1, rows 96:128 = hb0.
  - 4 matmuls per wave, one per GATE (i, f, g, o), each K=128 x M=128 x N=256.
    Stationary M-columns pack all four cells in order [b1, f0, f1, b0]
    (32 cols each), zero-padded so each cell contracts only its input rows.
    Bias enters via the ones row; the g-gate weights are pre-scaled by 2 so a
    single merged Sigmoid yields sigma(2g), and tanh(g) = 2*sigma(2g)-1.
  - Software-skewed pipeline: wave w computes f0@w, f1@w-1, b0@w-1, b1@w-2,
    which makes all four cells' gates computable from one packed snapshot.
  - x/xb rows are host-interleaved into one [pair, 2, 2, 256] DRAM buffer so
    each wave needs a single dynamic-offset DMA (SP registers are scarce).
  - Output projection: two M=1 matmuls (over packed, and over hb1) accumulate
    out[t] in PSUM; evacuated every 8 waves via DVE copies to a DRAM scratch
    in [t, b] layout; transposed to [b, t] with PE transposes at the end.
"""

import sys

sys.path.insert(0, "/opt/trn_rl_repo")

import numpy as np
import concourse.bass as bass
import concourse.tile as tile
import concourse.mybir as mybir
import bass_rust
from concourse.bass_utils import run_bass_kernel_spmd

S, B, H = 1024, 2048, 32
NCORES = 8
BL = B // NCORES  # 256

F32 = mybir.dt.float32
AF = mybir.ActivationFunctionType
OP = mybir.AluOpType

# cell order along M-columns / h_all / c_all partitions
# [f0: 0:32, f1: 32:64, b0: 64:96, b1: 96:128]
CELL_COL = {"f0": 0, "f1": 32, "b0": 64, "b1": 96}
# packed-state row blocks (X block lives in the top quadrant)
ROW_HF0, ROW_HF1, ROW_HB0 = 0, 32, 64
ROW_X, ROW_XB, ROW_ONES = 96, 97, 98


def _split_excess_waits(nc, max_waits=1):
    """walrus codegen in this toolchain supports only one sync-wait per
    instruction; split extras onto inserted wait-only drains."""
    n = 0
    for f in nc.m.functions:
        for bb in f.blocks:
            newl = []
            dirty = False
            for ins in bb.instructions:
                si = ins.sync_info
                waits = list(si.on_wait) if si is not None else []
                if len(waits) > max_waits:
                    dirty = True
                    k = len(waits) - max_waits
                    i = 0
                    while i < k:
                        chunk = waits[i : min(i + max_waits, k)]
                        d = mybir.InstDrain(name=f"zwsplit-{n}", is_reset_sema=False)
                        n += 1
                        d.engine = ins.engine
                        d.sync_info = bass_rust.SyncInfo(on_wait=chunk, on_update=[])
                        newl.append(d)
                        i += max_waits
                    si.on_wait = waits[k:]
                    ins.sync_info = si
                newl.append(ins)
            if dirty:
                bb.instructions = newl
    return n


def _gate_block(Wmat, gi):
    """rows of a torch 4H-row weight/bias for gate gi (torch order i,f,g,o)."""
    return Wmat[gi * H : (gi + 1) * H]


def build_weights(Wih_f0, Whh_f0, b_f0, Wih_f1, Whh_f1, b_f1,
                  Wih_b0, Whh_b0, b_b0, Wih_b1, Whh_b1, b_b1, Wlin, blin):
    """Pack per-gate stationary matrices Wg [gate, K=128, M=128] plus the two
    output-projection columns."""
    Wg = np.zeros((4, 128, 128), np.float32)
    for gi in range(4):
        sc = 2.0 if gi == 2 else 1.0  # tanh-gate pre-scale
        # --- cell f0: inp = x (row 0), h = hf0 (rows 32:64)
        c = CELL_COL["f0"]
        Wg[gi, ROW_X, c : c + H] = _gate_block(Wih_f0, gi)[:, 0] * sc
        Wg[gi, ROW_ONES, c : c + H] = _gate_block(b_f0, gi) * sc
        Wg[gi, ROW_HF0 : ROW_HF0 + H, c : c + H] = _gate_block(Whh_f0, gi).T * sc
        # --- cell f1: inp = hf0 (rows 32:64), h = hf1 (rows 64:96)
        c = CELL_COL["f1"]
        Wg[gi, ROW_ONES, c : c + H] = _gate_block(b_f1, gi) * sc
        Wg[gi, ROW_HF0 : ROW_HF0 + H, c : c + H] = _gate_block(Wih_f1, gi).T * sc
        Wg[gi, ROW_HF1 : ROW_HF1 + H, c : c + H] = _gate_block(Whh_f1, gi).T * sc
        # --- cell b0: inp = xb (row 1), h = hb0 (rows 96:128)
        c = CELL_COL["b0"]
        Wg[gi, ROW_XB, c : c + H] = _gate_block(Wih_b0, gi)[:, 0] * sc
        Wg[gi, ROW_ONES, c : c + H] = _gate_block(b_b0, gi) * sc
        Wg[gi, ROW_HB0 : ROW_HB0 + H, c : c + H] = _gate_block(Whh_b0, gi).T * sc
        # --- cell b1: inp = hb0 (rows 96:128), h-arg = hf1 (rows 64:96)
        c = CELL_COL["b1"]
        Wg[gi, ROW_ONES, c : c + H] = _gate_block(b_b1, gi) * sc
        Wg[gi, ROW_HB0 : ROW_HB0 + H, c : c + H] = _gate_block(Wih_b1, gi).T * sc
        Wg[gi, ROW_HF1 : ROW_HF1 + H, c : c + H] = _gate_block(Whh_b1, gi).T * sc

    # out = [hf1, hb1] @ Wlin.T + blin : wout1 over packed (hf1 + bias),
    # wout2 over hb1 tile.
    wout1 = np.zeros((128, 1), np.float32)
    wout1[ROW_ONES, 0] = blin[0]
    wout1[ROW_HF1 : ROW_HF1 + H, 0] = Wlin[0, 0:H]
    wout2 = np.zeros((128, 1), np.float32)
    wout2[96:128, 0] = Wlin[0, H : 2 * H]
    return Wg, wout1, wout2


def build_xpair(x_shard, s):
    """Interleave per-wave x rows: xp[p, r, d, :] is the row for packed
    partition r (0 = x, 1 = xb, 2 = ones) of wave w = 2p + d."""
    bl = x_shard.shape[1]
    npair = s // 2 + 1  # waves 0 .. s+1
    xp = np.zeros((npair, 3, 2, bl), np.float32)
    xp[:, 2, :, :] = 1.0
    for p in range(npair):
        for d in range(2):
            w = 2 * p + d
            if w < s:
                xp[p, 0, d] = x_shard[w]
            if w >= 1:
                xp[p, 1, d] = x_shard[(s + 1 - w) % s]
    return xp


def build_nc(s=S, unroll_pairs=8, dbg=False):
    assert (s // 2) % unroll_pairs == 0 and s % 2 == 0
    npair = s // 2 + 1
    nc = bass.Bass("TRN2", target_bir_lowering=False, debug=False,
                   num_devices=NCORES)

    xd = nc.declare_dram_parameter("xpair", [npair, 3, 2, BL], F32, isOutput=False)
    wgd = nc.declare_dram_parameter("Wg", [128, 4, 128], F32, isOutput=False)
    wo1d = nc.declare_dram_parameter("wout1", [128, 1], F32, isOutput=False)
    wo2d = nc.declare_dram_parameter("wout2", [128, 1], F32, isOutput=False)
    idd = nc.declare_dram_parameter("ident", [128, 128], F32, isOutput=False)
    outd = nc.declare_dram_parameter("out", [BL, s], F32, isOutput=True)
    oscr = nc.dram_tensor("oscr", [s // 2, 2, BL], F32)
    if dbg:
        dbg_w0 = nc.declare_dram_parameter("dbg_w0", [3, 128, 4 * BL], F32, isOutput=True)
        dbg_oscr = nc.declare_dram_parameter("dbg_oscr", [s // 2, 2, BL], F32, isOutput=True)
        dbg_pkq = nc.declare_dram_parameter("dbg_pkq", [128, 4, BL], F32, isOutput=True)
        dbg_call = nc.declare_dram_parameter("dbg_call", [128, BL], F32, isOutput=True)
        dbg_hb1 = nc.declare_dram_parameter("dbg_hb1", [2, 128, BL], F32, isOutput=True)
        dbg_sig = nc.declare_dram_parameter("dbg_sig", [2, 128, 4 * BL], F32, isOutput=True)

    with tile.TileContext(nc) as tc:
        with (
            tc.tile_pool(name="const", bufs=1) as cpool,
            tc.tile_pool(name="state", bufs=1) as spool,
            tc.tile_pool(name="psum", bufs=1, space="PSUM") as ppool,
        ):
            wg_t = cpool.tile([128, 4, 128], F32)
            wo1_t = cpool.tile([128, 1], F32)
            wo2_t = cpool.tile([128, 1], F32)
            id_t = cpool.tile([128, 128], F32)
            nc.sync.dma_start(wg_t[:], wgd[:])
            nc.sync.dma_start(wo1_t[:], wo1d[:])
            nc.sync.dma_start(wo2_t[:], wo2d[:])
            nc.sync.dma_start(id_t[:], idd[:])

            pkq = spool.tile([128, 4, BL], F32, name="pkq")
            c_all = spool.tile([128, BL], F32, name="c_all")
            sig = [spool.tile([128, 4 * BL], F32, name=f"sig{j}") for j in range(2)]
            wt = [spool.tile([128, BL], F32, name=f"wt{j}") for j in range(2)]
            u = [spool.tile([128, BL], F32, name=f"u{j}") for j in range(2)]
            v = [spool.tile([128, BL], F32, name=f"v{j}") for j in range(2)]
            tct = [spool.tile([128, BL], F32, name=f"tct{j}") for j in range(2)]
            hb1 = [spool.tile([128, BL], F32, name=f"hb1{j}") for j in range(2)]
            osb = [spool.tile([1, 8 * BL], F32, name=f"osb{j}") for j in range(2)]

            gps = ppool.tile([128, 4 * BL], F32, name="gps")
            ops_ = [ppool.tile([1, 4 * BL], F32, name=f"ops{j}") for j in range(2)]

            # ---- init ----
            nc.vector.memset(pkq[:], 0.0)
            nc.vector.memset(c_all[:], 0.0)
            for j in range(2):
                nc.vector.memset(hb1[j][:], 0.0)

            def wave(w_mod, do_out, oslot=None):
                """One wave. w_mod: python int with the wave's value mod 16."""
                j = w_mod % 4
                bb = w_mod % 2
                sg = sig[bb]
                p = pkq[:, j, :]
                for g in range(4):
                    nc.tensor.matmul(
                        gps[:, g * BL : (g + 1) * BL],
                        wg_t[:, g, :],
                        p,
                        start=True,
                        stop=True,
                    )
                if do_out:
                    ob, osl = oslot
                    nc.tensor.matmul(
                        ops_[ob][0:1, osl * BL : (osl + 1) * BL],
                        wo1_t[:],
                        p,
                        start=True,
                        stop=False,
                    )
                nc.scalar.activation(sg[:], gps[:], AF.Sigmoid)
                i_s = sg[:, 0:BL]
                f_s = sg[:, BL : 2 * BL]
                g_s = sg[:, 2 * BL : 3 * BL]
                # o_s = sg[:, 3*BL:4*BL]
                # tanh(g) = 2*sigma(2g) - 1
                nc.vector.tensor_scalar(wt[bb][:], g_s, 2.0, -1.0, OP.mult, OP.add)
                nc.gpsimd.tensor_tensor(v[bb][:], f_s, c_all[:], OP.mult)
                nc.vector.tensor_tensor(u[bb][:], i_s, wt[bb][:], OP.mult)
                nc.vector.tensor_tensor(c_all[:], u[bb][:], v[bb][:], OP.add)
                nc.scalar.activation(tct[bb][:], c_all[:], AF.Tanh)
                # h2a: cells f0,f1,b0 (partitions 0:96) -> next packed h rows
                nc.vector.tensor_tensor(
                    pkq[0:96, (w_mod + 1) % 4, :],
                    sg[0:96, 3 * BL : 4 * BL],
                    tct[bb][0:96, :],
                    OP.mult,
                )
                # h2b: cell b1 (partitions 96:128) -> hb1 rows 96:128
                nc.gpsimd.tensor_tensor(
                    hb1[bb][96:128, :],
                    sg[96:128, 3 * BL : 4 * BL],
                    tct[bb][96:128, :],
                    OP.mult,
                )
                if do_out:
                    ob, osl = oslot
                    nc.tensor.matmul(
                        ops_[ob][0:1, osl * BL : (osl + 1) * BL],
                        wo2_t[:],
                        hb1[bb][:],
                        start=False,
                        stop=True,
                    )

            # ---- prologue: waves 0 and 1 (no out-proj; cleanup memsets) ----
            nc.sync.dma_start(pkq[96:99, 0:2, :], xd[0:1, :, :, :])
            wave(0, do_out=False)
            if dbg:
                nc.sync.dma_start(dbg_w0[0], sig[0][:])
                nc.sync.dma_start(dbg_w0[1, :, 0:BL], c_all[:])
                nc.sync.dma_start(dbg_w0[1, :, BL:2*BL], tct[0][:])
                nc.sync.dma_start(dbg_w0[1, :, 2*BL:3*BL], wt[0][:])
                nc.sync.dma_start(dbg_w0[1, :, 3*BL:4*BL], u[0][:])
                nc.sync.dma_start(dbg_w0[2, :, 0:4*BL], pkq[:].rearrange("p j b -> p (j b)") if hasattr(pkq[:], "rearrange") else pkq[:])
            # zero junk written into hf1/hb0 slots of pk slot 1 and cf1/cb0 rows
            nc.vector.memset(pkq[32:64, 1, :], 0.0)
            nc.vector.memset(pkq[64:96, 1, :], 0.0)
            nc.vector.memset(c_all[32:64, :], 0.0)
            nc.vector.memset(c_all[64:96, :], 0.0)
            wave(1, do_out=False)
            # zero junk cb1 rows (cell b1 = rows 96:128 of c_all)
            nc.vector.memset(c_all[96:128, :], 0.0)

            # ---- main loop: pairs 1 .. s//2, waves 2 .. s+1 ----
            with tc.For_i(1, s // 2 + 1, unroll_pairs) as ip:
                for k in range(unroll_pairs):
                    # pair p = ip + k covers waves w = 2p, 2p+1
                    j0 = (2 + 2 * k) % 4
                    nc.sync.dma_start(
                        pkq[96:99, j0 : j0 + 2, :], xd[bass.ds(ip + k, 1), :, :, :]
                    )
                    for d in range(2):
                        w_mod = (2 + 2 * k + d) % 16
                        idx = (2 * k + d) % 8  # out slot index
                        wave(w_mod, do_out=True, oslot=(idx // 4, idx % 4))
                        if idx == 7:
                            half = (2 * k + d) // 8  # 0 or 1 within the body
                            ob2 = half  # which osb buffer
                            nc.vector.tensor_copy(
                                osb[ob2][0:1, 0 : 4 * BL], ops_[0][:]
                            )
                            nc.vector.tensor_copy(
                                osb[ob2][0:1, 4 * BL : 8 * BL], ops_[1][:]
                            )
                            nc.sync.dma_start(
                                oscr[bass.ds(ip - 1 + 4 * half, 4), :, :],
                                osb[ob2][:],
                            )

            if dbg:
                nc.sync.dma_start(dbg_oscr[:], oscr[:])
                nc.sync.dma_start(dbg_pkq[:], pkq[:])
                nc.sync.dma_start(dbg_call[:], c_all[:])
                for jj in range(2):
                    nc.sync.dma_start(dbg_hb1[jj], hb1[jj][:])
                    nc.sync.dma_start(dbg_sig[jj], sig[jj][:])

            # ---- end phase: transpose oscr [t, b] -> out [b, t] ----
            nchunk = s // 128
            if nchunk == 0:
                nchunk = None  # debug-small runs skip the transpose phase
            if nchunk is None:
                nchunk = 0
                outsb = None
                stg = []
            else:
                outsb = spool.tile([128, 2, nchunk, 128], F32, name="outsb")
                stg = [spool.tile([128, BL], F32, name=f"stg{j}") for j in range(2)]
            for c in range(nchunk):
                st = stg[c % 2]
                nc.sync.dma_start(st[:], oscr[c * 64 : (c + 1) * 64, :, :])
                for g in range(2):
                    tp = gps[:, 0:128] if g == 0 else gps[:, 128:256]
                    nc.tensor.transpose(tp, st[:, g * 128 : (g + 1) * 128], id_t[:])
                    nc.vector.tensor_copy(outsb[:, g, c, :], tp)
            if outsb is not None:
                nc.sync.dma_start(outd[0:128, :], outsb[:, 0, :, :])
                nc.sync.dma_start(outd[128:256, :], outsb[:, 1, :, :])

    _split_excess_waits(nc)
    return nc


_NC_CACHE = {}


def _get_nc(s=S, unroll_pairs=8, dbg=False):
    key = (s, unroll_pairs, dbg)
    if key not in _NC_CACHE:
        _NC_CACHE[key] = build_nc(s, unroll_pairs, dbg)
    return _NC_CACHE[key]


def run(x, weights, s=S, unroll_pairs=8, dbg=False, trace=False):
    """x: [s, B] fp32 (already squeezed); weights: dict of reference arrays."""
    Wg, wout1, wout2 = build_weights(**weights)
    nc = _get_nc(s, unroll_pairs, dbg)
    ident = np.eye(128, dtype=np.float32)
    in_maps = []
    for c in range(NCORES):
        xs = np.ascontiguousarray(x[:, c * BL : (c + 1) * BL])
        in_maps.append(
            {"xpair": build_xpair(xs, s),
             "Wg": np.ascontiguousarray(Wg.transpose(1, 0, 2)),
             "wout1": wout1, "wout2": wout2, "ident": ident}
        )
    res = run_bass_kernel_spmd(nc, in_maps, list(range(NCORES)), trace=trace)
    out = np.concatenate([res.results[c]["out"] for c in range(NCORES)], axis=0)
    return out, res


def kernel(x, Wih_f0, Whh_f0, b_f0, Wih_f1, Whh_f1, b_f1,
           Wih_b0, Whh_b0, b_b0, Wih_b1, Whh_b1, b_b1, Wlin, blin, future):
    assert int(future) == 0, "kernel hardcodes future=0"
    x = np.asarray(x, np.float32)
    s, b, _ = x.shape
    assert (s, b) == (S, B)
    weights = dict(
        Wih_f0=np.asarray(Wih_f0, np.float32), Whh_f0=np.asarray(Whh_f0, np.float32),
        b_f0=np.asarray(b_f0, np.float32),
        Wih_f1=np.asarray(Wih_f1, np.float32), Whh_f1=np.asarray(Whh_f1, np.float32),
        b_f1=np.asarray(b_f1, np.float32),
        Wih_b0=np.asarray(Wih_b0, np.float32), Whh_b0=np.asarray(Whh_b0, np.float32),
        b_b0=np.asarray(b_b0, np.float32),
        Wih_b1=np.asarray(Wih_b1, np.float32), Whh_b1=np.asarray(Whh_b1, np.float32),
        b_b1=np.asarray(b_b1, np.float32),
        Wlin=np.asarray(Wlin, np.float32), blin=np.asarray(blin, np.float32),
    )
    out, _ = run(x[:, :, 0], weights, s=S)
    return out



# revision 10
# speedup vs baseline: 2.1062x; 2.1062x over previous
"""Bidirectional 2-layer LSTM (with replicated hf1-input bug) + per-step linear,
as a Trainium2 Bass/Tile kernel, data-parallel over batch across 8 NeuronCores.

v2: fp16 datapath + two phase-shifted half-batch pipelines per core.

Layout strategy (per core, B_loc=256 batch split into halves A/B of 128):
  - packed state tile pkq [128 rows, 4 slots, 2 halves, 128 batch] fp16:
      rows 0:32 hf0, 32:64 hf1, 64:96 hb0; row 96 = x[t], 97 = xb[t],
      98 = ones (bias enters via the ones row).
  - per half-wave: 4 fp16 matmuls (one per gate i,f,g,o), K=128 x M=128 x
    N=128, into a per-half PSUM bank; the g-gate weights are pre-scaled by 2
    so one merged Sigmoid yields sigma(2g), and tanh(g) = 2*sigma(2g)-1.
  - Scalar engine: one Sigmoid over [128, 512] + one Tanh over the fp16 cell
    state [128, 128] per half-wave.  This is the bottleneck engine; all other
    work overlaps under it because the two half-batch chains are interleaved
    (phase1 A, phase1 B, phase2 A, phase2 B) so every in-order engine queue
    always has ready work from the other half.
  - DVE: v = sig_f*c ; c = 2*u3 + v (scalar_tensor_tensor); h2a = sig_o *
    tanh(c) for cells f0/f1/b0 -> next pkq slot.  GpSimd: u3 =
    (sig_2g-0.5)*sig_i and h2b for cell b1 (feeds out-proj only).
  - Output projection: wout1 over packed (hf1 + bias, start) and wout2 over
    hb1 (stop) accumulate out[t] per half into an 8-slot PSUM ring; wout2 is
    emitted one wave late so the PE queue never waits on GpSimd.  Every 4
    waves the older 4 slots go PSUM->SBUF (DVE copy) -> DRAM scratch in
    [t, half, b] layout; PE-transposed to [b, t] at the end.
"""

import sys

sys.path.insert(0, "/opt/trn_rl_repo")

import numpy as np
import concourse.bass as bass
import concourse.tile as tile
import concourse.mybir as mybir
import bass_rust
from concourse.bass_utils import run_bass_kernel_spmd

S, B, H = 1024, 2048, 32
NCORES = 8
BL = B // NCORES  # 256 per-core batch
HB = BL // 2      # 128 half-batch

F32 = mybir.dt.float32
F16 = mybir.dt.float16
AF = mybir.ActivationFunctionType
OP = mybir.AluOpType

# cell order along M-columns / state partitions: [f0, f1, b0, b1]
CELL_COL = {"f0": 0, "f1": 32, "b0": 64, "b1": 96}
ROW_HF0, ROW_HF1, ROW_HB0 = 0, 32, 64
ROW_X, ROW_XB, ROW_ONES = 96, 97, 98


def _split_excess_waits(nc, max_waits=1):
    """walrus codegen in this toolchain supports only one sync-wait per
    instruction; split extras onto inserted wait-only drains."""
    n = 0
    for f in nc.m.functions:
        for bb in f.blocks:
            newl = []
            dirty = False
            for ins in bb.instructions:
                si = ins.sync_info
                waits = list(si.on_wait) if si is not None else []
                if len(waits) > max_waits:
                    dirty = True
                    k = len(waits) - max_waits
                    i = 0
                    while i < k:
                        chunk = waits[i : min(i + max_waits, k)]
                        d = mybir.InstDrain(name=f"zwsplit-{n}", is_reset_sema=False)
                        n += 1
                        d.engine = ins.engine
                        d.sync_info = bass_rust.SyncInfo(on_wait=chunk, on_update=[])
                        newl.append(d)
                        i += max_waits
                    si.on_wait = waits[k:]
                    ins.sync_info = si
                newl.append(ins)
            if dirty:
                bb.instructions = newl
    return n


def _gate_block(Wmat, gi):
    """rows of a torch 4H-row weight/bias for gate gi (torch order i,f,g,o)."""
    return Wmat[gi * H : (gi + 1) * H]


def build_weights(Wih_f0, Whh_f0, b_f0, Wih_f1, Whh_f1, b_f1,
                  Wih_b0, Whh_b0, b_b0, Wih_b1, Whh_b1, b_b1, Wlin, blin):
    """Pack per-gate stationary matrices Wg -> [K=128, gate, M=128] plus the
    two output-projection columns (all fp16)."""
    Wg = np.zeros((4, 128, 128), np.float32)
    for gi in range(4):
        sc = 2.0 if gi == 2 else 1.0  # tanh-gate pre-scale
        c = CELL_COL["f0"]  # inp = x, h = hf0
        Wg[gi, ROW_X, c : c + H] = _gate_block(Wih_f0, gi)[:, 0] * sc
        Wg[gi, ROW_ONES, c : c + H] = _gate_block(b_f0, gi) * sc
        Wg[gi, ROW_HF0 : ROW_HF0 + H, c : c + H] = _gate_block(Whh_f0, gi).T * sc
        c = CELL_COL["f1"]  # inp = hf0, h = hf1
        Wg[gi, ROW_ONES, c : c + H] = _gate_block(b_f1, gi) * sc
        Wg[gi, ROW_HF0 : ROW_HF0 + H, c : c + H] = _gate_block(Wih_f1, gi).T * sc
        Wg[gi, ROW_HF1 : ROW_HF1 + H, c : c + H] = _gate_block(Whh_f1, gi).T * sc
        c = CELL_COL["b0"]  # inp = xb, h = hb0
        Wg[gi, ROW_XB, c : c + H] = _gate_block(Wih_b0, gi)[:, 0] * sc
        Wg[gi, ROW_ONES, c : c + H] = _gate_block(b_b0, gi) * sc
        Wg[gi, ROW_HB0 : ROW_HB0 + H, c : c + H] = _gate_block(Whh_b0, gi).T * sc
        c = CELL_COL["b1"]  # inp = hb0, h-arg = hf1 (replicated bug)
        Wg[gi, ROW_ONES, c : c + H] = _gate_block(b_b1, gi) * sc
        Wg[gi, ROW_HB0 : ROW_HB0 + H, c : c + H] = _gate_block(Wih_b1, gi).T * sc
        Wg[gi, ROW_HF1 : ROW_HF1 + H, c : c + H] = _gate_block(Whh_b1, gi).T * sc

    wout1 = np.zeros((128, 1), np.float32)
    wout1[ROW_ONES, 0] = blin[0]
    wout1[ROW_HF1 : ROW_HF1 + H, 0] = Wlin[0, 0:H]
    wout2 = np.zeros((128, 1), np.float32)
    wout2[96:128, 0] = Wlin[0, H : 2 * H]
    return (np.ascontiguousarray(Wg.transpose(1, 0, 2)).astype(np.float16),
            wout1.astype(np.float16), wout2.astype(np.float16))


def build_xpair(x_shard, s):
    """Interleave per-wave x rows: xp[p, r, d, h, :] is the row for packed
    partition 96+r (0 = x, 1 = xb, 2 = ones) of wave w = 2p + d, half h."""
    bl = x_shard.shape[1]
    hb = bl // 2
    npair = s // 2 + 2  # one pad pair for the loop's depth-1 prefetch
    xp = np.zeros((npair, 3, 2, 2, hb), np.float16)
    xp[:, 2] = 1.0
    x16 = x_shard.astype(np.float16)
    xp[0 : s // 2, 0] = x16.reshape(s // 2, 2, 2, hb)
    # xb rows: wave w in 1..s+1 reads x[(s + 1 - w) % s]
    w = np.arange(1, s + 2)
    xb = x16[(s + 1 - w) % s].reshape(-1, 2, hb)  # [s+1, 2, hb]
    xbp = np.zeros((npair * 2, 2, hb), np.float16)
    xbp[1 : s + 2] = xb
    xp[:, 1] = xbp.reshape(npair, 2, 2, hb)
    return xp


def build_nc(s=S, dbg=False, split_waits=True):
    assert s % 128 == 0 and (s // 2 - 4) % 4 == 0
    nc = bass.Bass("TRN2", target_bir_lowering=False, debug=False,
                   num_devices=NCORES)

    npair = s // 2 + 2
    xd = nc.declare_dram_parameter("xpair", [npair, 3, 2, 2, HB], F16, isOutput=False)
    wgd = nc.declare_dram_parameter("Wg", [128, 4, 128], F16, isOutput=False)
    wo1d = nc.declare_dram_parameter("wout1", [128, 1], F16, isOutput=False)
    wo2d = nc.declare_dram_parameter("wout2", [128, 1], F16, isOutput=False)
    idd = nc.declare_dram_parameter("ident", [128, 128], F32, isOutput=False)
    outd = nc.declare_dram_parameter("out", [BL, s], F32, isOutput=True)
    # [pair, d, half, hb] == row-major [t, half, hb]
    oscr = nc.dram_tensor("oscr", [s // 2, 2, 2, HB], F32)

    with tile.TileContext(nc) as tc:
        with (
            tc.tile_pool(name="const", bufs=1) as cpool,
            tc.tile_pool(name="state", bufs=1) as spool,
            tc.tile_pool(name="psum", bufs=1, space="PSUM") as ppool,
        ):
            wg_t = cpool.tile([128, 4, 128], F16)
            wo1_t = cpool.tile([128, 1], F16)
            wo2_t = cpool.tile([128, 1], F16)
            id_t = cpool.tile([128, 128], F32)
            nc.sync.dma_start(wg_t[:], wgd[:])
            nc.sync.dma_start(wo1_t[:], wo1d[:])
            nc.sync.dma_start(wo2_t[:], wo2d[:])
            nc.sync.dma_start(id_t[:], idd[:])

            pkq = spool.tile([128, 4, 2, HB], F16, name="pkq")
            c_t = spool.tile([128, 2, HB], F16, name="c_t")
            sig = spool.tile([128, 2, 4 * HB], F16, name="sig")
            tct = spool.tile([128, 2, HB], F16, name="tct")
            u3 = spool.tile([128, 2, HB], F16, name="u3")
            v_t = spool.tile([128, 2, HB], F16, name="v_t")
            hb1 = spool.tile([128, 2, 2, HB], F16, name="hb1")
            osb = spool.tile([1, 2, 2, 4 * HB], F32, name="osb")

            gps = ppool.tile([128, 2, 4 * HB], F32, name="gps")
            ops_ = ppool.tile([1, 2, 8 * HB], F32, name="ops")

            # ---- init ----
            nc.vector.memset(pkq[:], 0.0)
            nc.vector.memset(c_t[:], 0.0)
            nc.vector.memset(hb1[:], 0.0)

            def phase1(wm, h, do_out, do_wo2):
                """wm = wave index mod 8."""
                j = wm % 4
                p = pkq[:, j, h, :]
                if do_wo2:  # deferred wout2 for wave w-1
                    sl = (wm - 1) % 8
                    nc.tensor.matmul(
                        ops_[0:1, h, sl * HB : (sl + 1) * HB], wo2_t[:],
                        hb1[:, h, (wm - 1) % 2, :],
                        start=False, stop=True, skip_group_check=True,
                    )
                for g in range(4):
                    nc.tensor.matmul(
                        gps[:, h, g * HB : (g + 1) * HB],
                        wg_t[:, g, :], p, start=True, stop=True,
                    )
                if do_out:
                    nc.tensor.matmul(
                        ops_[0:1, h, wm * HB : (wm + 1) * HB], wo1_t[:], p,
                        start=True, stop=False, skip_group_check=True,
                    )
                nc.scalar.activation(sig[:, h, :], gps[:, h, :], AF.Sigmoid)
                i_s = sig[:, h, 0:HB]
                f_s = sig[:, h, HB : 2 * HB]
                g_s = sig[:, h, 2 * HB : 3 * HB]
                # u3 = (sigma(2g) - 0.5) * sigma(i)  == tanh(g)*sigma(i)/2
                nc.vector.scalar_tensor_tensor(
                    u3[:, h, :], g_s, -0.5, i_s, OP.add, OP.mult)
                nc.vector.tensor_tensor(v_t[:, h, :], f_s, c_t[:, h, :], OP.mult)
                nc.vector.scalar_tensor_tensor(
                    c_t[:, h, :], u3[:, h, :], 2.0, v_t[:, h, :], OP.mult, OP.add)

            def phase2(wm, h):
                j = wm % 4
                nc.scalar.activation(tct[:, h, :], c_t[:, h, :], AF.Tanh)
                nc.vector.tensor_tensor(
                    pkq[0:96, (j + 1) % 4, h, :],
                    sig[0:96, h, 3 * HB : 4 * HB],
                    tct[0:96, h, :], OP.mult,
                )
                nc.gpsimd.tensor_tensor(
                    hb1[96:128, h, wm % 2, :],
                    sig[96:128, h, 3 * HB : 4 * HB],
                    tct[96:128, h, :], OP.mult,
                )

            def evac4(wm, h, dst):
                """copy the older 4 ops slots to SBUF and DMA out.
                At wm==0: slots 4..7 (buf 1); at wm==4: slots 0..3 (buf 0)."""
                base, buf = (4, 1) if wm == 0 else (0, 0)
                nc.vector.tensor_copy(
                    osb[0:1, h, buf, :],
                    ops_[0:1, h, base * HB : (base + 4) * HB])
                nc.sync.dma_start(dst, osb[0:1, h, buf, :])

            def xload(pair, j0):
                nc.sync.dma_start(
                    pkq[96:99, j0 : j0 + 2, :, :],
                    xd[pair : pair + 1, :, :, :, :]
                    if isinstance(pair, int)
                    else xd[pair, :, :, :, :])

            def full_wave(wm, do_out, do_wo2, ed=None, peel23=False):
                for h in range(2):
                    phase1(wm, h, do_out, do_wo2)
                    if peel23:
                        # waves 2,3 (slots 2,3) -> rows 0,1
                        nc.vector.tensor_copy(
                            osb[0:1, h, 0, 0 : 2 * HB],
                            ops_[0:1, h, 2 * HB : 4 * HB])
                        nc.sync.dma_start(oscr[0:1, :, h, :],
                                          osb[0:1, h, 0, 0 : 2 * HB])
                    elif ed is not None:
                        evac4(wm, h, ed(h))
                for h in range(2):
                    phase2(wm, h)

            # ---- prologue: waves 0,1 (skew priming, no out) ----
            xload(0, 0)
            xload(1, 2)
            full_wave(0, do_out=False, do_wo2=False)
            # zero junk written into hf1/hb0 rows of pkq slot 1 and cf1/cb0
            nc.vector.memset(pkq[32:64, 1, :, :], 0.0)
            nc.vector.memset(pkq[64:96, 1, :, :], 0.0)
            nc.vector.memset(c_t[32:64, :, :], 0.0)
            nc.vector.memset(c_t[64:96, :, :], 0.0)
            full_wave(1, do_out=False, do_wo2=False)
            nc.vector.memset(c_t[96:128, :, :], 0.0)
            xload(2, 0)

            # ---- peel: waves 2..9 (pairs 1..4) ----
            for w in range(2, 10):
                wm = w % 8
                if w % 2 == 0 and w >= 4:
                    p = w // 2  # current pair: prefetch pair p+1
                    xload(p + 1, 0 if (p + 1) % 2 == 0 else 2)
                ed = None
                if w == 8:
                    # waves 4..7 (slots 4..7) -> rows 2..5 (pairs 1,2)
                    def ed(h):
                        return oscr[1:3, :, h, :]
                full_wave(wm, do_out=True, do_wo2=(w >= 3),
                          ed=ed, peel23=(w == 4))

            # ---- main loop: pairs 5 .. s//2, waves 10 .. s+1 ----
            with tc.For_i(5, s // 2 + 1, 4) as ip:
                for k in range(4):
                    # pair p = ip + k covers waves 2p, 2p+1
                    # (ip = 1 mod 4) -> wave mod 8 = 2+2k, 3+2k
                    xload(bass.ds(ip + k + 1, 1), 0 if k % 2 == 0 else 2)
                    for d in range(2):
                        wm = (2 + 2 * k + d) % 8
                        ed = None
                        if wm == 4:  # w = 2ip+2: slots 0..3 -> pairs ip-2, ip-1
                            def ed(h, _ip=ip):
                                return oscr[bass.ds(_ip - 2, 2), :, h, :]
                        elif wm == 0:  # w = 2ip+6: slots 4..7 -> pairs ip, ip+1
                            def ed(h, _ip=ip):
                                return oscr[bass.ds(_ip, 2), :, h, :]
                        full_wave(wm, do_out=True, do_wo2=True, ed=ed)

            # ---- epilogue: wo2 for wave s+1; evac waves s, s+1 (slots 0,1) --
            for h in range(2):
                # (s+1) % 8 == 1 for s % 8 == 0; hb1 ring (s+1) % 2 == 1
                nc.tensor.matmul(
                    ops_[0:1, h, HB : 2 * HB], wo2_t[:], hb1[:, h, 1, :],
                    start=False, stop=True, skip_group_check=True)
                nc.vector.tensor_copy(
                    osb[0:1, h, 0, 0 : 2 * HB], ops_[0:1, h, 0 : 2 * HB])
                nc.sync.dma_start(oscr[s // 2 - 1 : s // 2, :, h, :],
                                  osb[0:1, h, 0, 0 : 2 * HB])

            # ---- end phase: transpose oscr [t, half, b2] -> out [b, t] ----
            nchunk = s // 128
            outsb = spool.tile([128, 2, nchunk, 128], F32, name="outsb")
            stg = [spool.tile([128, 2, HB], F32, name=f"stg{j}") for j in range(2)]
            for ci in range(nchunk):
                st = stg[ci % 2]
                nc.sync.dma_start(st[:], oscr[ci * 64 : (ci + 1) * 64, :, :, :])
                for g in range(2):
                    tp = gps[:, g, 0:128]
                    nc.tensor.transpose(tp, st[:, g, :], id_t[:])
                    nc.vector.tensor_copy(outsb[:, g, ci, :], tp)
            nc.sync.dma_start(outd[0:HB, :], outsb[:, 0, :, :])
            nc.sync.dma_start(outd[HB : 2 * HB, :], outsb[:, 1, :, :])

    if split_waits:
        _split_excess_waits(nc)
    return nc


_NC_CACHE = {}


def _get_nc(s=S, dbg=False):
    key = (s, dbg)
    if key not in _NC_CACHE:
        _NC_CACHE[key] = build_nc(s, dbg)
    return _NC_CACHE[key]


def run(x, weights, s=S, dbg=False, trace=False):
    """x: [s, B] fp32 (already squeezed); weights: dict of reference arrays."""
    Wg, wout1, wout2 = build_weights(**weights)
    nc = _get_nc(s, dbg)
    ident = np.eye(128, dtype=np.float32)
    in_maps = []
    for c in range(NCORES):
        xs = np.ascontiguousarray(x[:, c * BL : (c + 1) * BL])
        in_maps.append(
            {"xpair": build_xpair(xs, s),
             "Wg": Wg, "wout1": wout1, "wout2": wout2, "ident": ident}
        )
    res = run_bass_kernel_spmd(nc, in_maps, list(range(NCORES)), trace=trace)
    out = np.concatenate([res.results[c]["out"] for c in range(NCORES)], axis=0)
    return out, res


def kernel(x, Wih_f0, Whh_f0, b_f0, Wih_f1, Whh_f1, b_f1,
           Wih_b0, Whh_b0, b_b0, Wih_b1, Whh_b1, b_b1, Wlin, blin, future):
    assert int(future) == 0, "kernel hardcodes future=0"
    x = np.asarray(x, np.float32)
    s, b, _ = x.shape
    assert (s, b) == (S, B)
    weights = dict(
        Wih_f0=np.asarray(Wih_f0, np.float32), Whh_f0=np.asarray(Whh_f0, np.float32),
        b_f0=np.asarray(b_f0, np.float32),
        Wih_f1=np.asarray(Wih_f1, np.float32), Whh_f1=np.asarray(Whh_f1, np.float32),
        b_f1=np.asarray(b_f1, np.float32),
        Wih_b0=np.asarray(Wih_b0, np.float32), Whh_b0=np.asarray(Whh_b0, np.float32),
        b_b0=np.asarray(b_b0, np.float32),
        Wih_b1=np.asarray(Wih_b1, np.float32), Whh_b1=np.asarray(Whh_b1, np.float32),
        b_b1=np.asarray(b_b1, np.float32),
        Wlin=np.asarray(Wlin, np.float32), blin=np.asarray(blin, np.float32),
    )
    out, _ = run(x[:, :, 0], weights, s=S)
    return out


# revision 14
# speedup vs baseline: 2.1278x; 1.0103x over previous
"""Bidirectional 2-layer LSTM (with replicated hf1-input bug) + per-step linear,
as a Trainium2 Bass/Tile kernel, data-parallel over batch across 8 NeuronCores.

v2: fp16 datapath + two phase-shifted half-batch pipelines per core.

Layout strategy (per core, B_loc=256 batch split into halves A/B of 128):
  - packed state tile pkq [128 rows, 4 slots, 2 halves, 128 batch] fp16:
      rows 0:32 hf0, 32:64 hf1, 64:96 hb0; row 96 = x[t], 97 = xb[t],
      98 = ones (bias enters via the ones row).
  - per half-wave: 4 fp16 matmuls (one per gate i,f,g,o), K=128 x M=128 x
    N=128, into a per-half PSUM bank; the g-gate weights are pre-scaled by 2
    so one merged Sigmoid yields sigma(2g), and tanh(g) = 2*sigma(2g)-1.
  - Scalar engine: one Sigmoid over [128, 512] + one Tanh over the fp16 cell
    state [128, 128] per half-wave.  This is the bottleneck engine; all other
    work overlaps under it because the two half-batch chains are interleaved
    (phase1 A, phase1 B, phase2 A, phase2 B) so every in-order engine queue
    always has ready work from the other half.
  - DVE: v = sig_f*c ; c = 2*u3 + v (scalar_tensor_tensor); h2a = sig_o *
    tanh(c) for cells f0/f1/b0 -> next pkq slot.  GpSimd: u3 =
    (sig_2g-0.5)*sig_i and h2b for cell b1 (feeds out-proj only).
  - Output projection: wout1 over packed (hf1 + bias, start) and wout2 over
    hb1 (stop) accumulate out[t] per half into an 8-slot PSUM ring; wout2 is
    emitted one wave late so the PE queue never waits on GpSimd.  Every 4
    waves the older 4 slots go PSUM->SBUF (DVE copy) -> DRAM scratch in
    [t, half, b] layout; PE-transposed to [b, t] at the end.
"""

import sys

sys.path.insert(0, "/opt/trn_rl_repo")

import numpy as np
import concourse.bass as bass
import concourse.tile as tile
import concourse.mybir as mybir
import bass_rust
from concourse.bass_utils import run_bass_kernel_spmd

S, B, H = 1024, 2048, 32
NCORES = 8
BL = B // NCORES  # 256 per-core batch
HB = BL // 2      # 128 half-batch

F32 = mybir.dt.float32
F16 = mybir.dt.float16
AF = mybir.ActivationFunctionType
OP = mybir.AluOpType

# cell order along M-columns / state partitions: [f0, f1, b0, b1]
CELL_COL = {"f0": 0, "f1": 32, "b0": 64, "b1": 96}
ROW_HF0, ROW_HF1, ROW_HB0 = 0, 32, 64
ROW_X, ROW_XB, ROW_ONES = 96, 97, 98


def _split_excess_waits(nc, max_waits=1):
    """walrus codegen in this toolchain supports only one sync-wait per
    instruction; split extras onto inserted wait-only drains."""
    n = 0
    for f in nc.m.functions:
        for bb in f.blocks:
            newl = []
            dirty = False
            for ins in bb.instructions:
                si = ins.sync_info
                waits = list(si.on_wait) if si is not None else []
                if len(waits) > max_waits:
                    dirty = True
                    k = len(waits) - max_waits
                    i = 0
                    while i < k:
                        chunk = waits[i : min(i + max_waits, k)]
                        d = mybir.InstDrain(name=f"zwsplit-{n}", is_reset_sema=False)
                        n += 1
                        d.engine = ins.engine
                        d.sync_info = bass_rust.SyncInfo(on_wait=chunk, on_update=[])
                        newl.append(d)
                        i += max_waits
                    si.on_wait = waits[k:]
                    ins.sync_info = si
                newl.append(ins)
            if dirty:
                bb.instructions = newl
    return n


def _gate_block(Wmat, gi):
    """rows of a torch 4H-row weight/bias for gate gi (torch order i,f,g,o)."""
    return Wmat[gi * H : (gi + 1) * H]


def build_weights(Wih_f0, Whh_f0, b_f0, Wih_f1, Whh_f1, b_f1,
                  Wih_b0, Whh_b0, b_b0, Wih_b1, Whh_b1, b_b1, Wlin, blin):
    """Pack per-gate stationary matrices Wg -> [K=128, gate, M=128] plus the
    two output-projection columns (all fp16)."""
    Wg = np.zeros((4, 128, 128), np.float32)
    for gi in range(4):
        sc = 2.0 if gi == 2 else 1.0  # tanh-gate pre-scale
        c = CELL_COL["f0"]  # inp = x, h = hf0
        Wg[gi, ROW_X, c : c + H] = _gate_block(Wih_f0, gi)[:, 0] * sc
        Wg[gi, ROW_ONES, c : c + H] = _gate_block(b_f0, gi) * sc
        Wg[gi, ROW_HF0 : ROW_HF0 + H, c : c + H] = _gate_block(Whh_f0, gi).T * sc
        c = CELL_COL["f1"]  # inp = hf0, h = hf1
        Wg[gi, ROW_ONES, c : c + H] = _gate_block(b_f1, gi) * sc
        Wg[gi, ROW_HF0 : ROW_HF0 + H, c : c + H] = _gate_block(Wih_f1, gi).T * sc
        Wg[gi, ROW_HF1 : ROW_HF1 + H, c : c + H] = _gate_block(Whh_f1, gi).T * sc
        c = CELL_COL["b0"]  # inp = xb, h = hb0
        Wg[gi, ROW_XB, c : c + H] = _gate_block(Wih_b0, gi)[:, 0] * sc
        Wg[gi, ROW_ONES, c : c + H] = _gate_block(b_b0, gi) * sc
        Wg[gi, ROW_HB0 : ROW_HB0 + H, c : c + H] = _gate_block(Whh_b0, gi).T * sc
        c = CELL_COL["b1"]  # inp = hb0, h-arg = hf1 (replicated bug)
        Wg[gi, ROW_ONES, c : c + H] = _gate_block(b_b1, gi) * sc
        Wg[gi, ROW_HB0 : ROW_HB0 + H, c : c + H] = _gate_block(Wih_b1, gi).T * sc
        Wg[gi, ROW_HF1 : ROW_HF1 + H, c : c + H] = _gate_block(Whh_b1, gi).T * sc

    wout1 = np.zeros((128, 1), np.float32)
    wout1[ROW_ONES, 0] = blin[0]
    wout1[ROW_HF1 : ROW_HF1 + H, 0] = Wlin[0, 0:H]
    wout2 = np.zeros((128, 1), np.float32)
    wout2[96:128, 0] = Wlin[0, H : 2 * H]
    return (np.ascontiguousarray(Wg.transpose(1, 0, 2)).astype(np.float16),
            wout1.astype(np.float16), wout2.astype(np.float16))


def build_xpair(x_shard, s):
    """Interleave per-wave x rows: xp[p, r, d, h, :] is the row for packed
    partition 96+r (0 = x, 1 = xb, 2 = ones) of wave w = 2p + d, half h."""
    bl = x_shard.shape[1]
    hb = bl // 2
    npair = s // 2 + 2  # one pad pair for the loop's depth-1 prefetch
    xp = np.zeros((npair, 3, 2, 2, hb), np.float16)
    xp[:, 2] = 1.0
    x16 = x_shard.astype(np.float16)
    xp[0 : s // 2, 0] = x16.reshape(s // 2, 2, 2, hb)
    # xb rows: wave w in 1..s+1 reads x[(s + 1 - w) % s]
    w = np.arange(1, s + 2)
    xb = x16[(s + 1 - w) % s].reshape(-1, 2, hb)  # [s+1, 2, hb]
    xbp = np.zeros((npair * 2, 2, hb), np.float16)
    xbp[1 : s + 2] = xb
    xp[:, 1] = xbp.reshape(npair, 2, 2, hb)
    return xp


def build_nc(s=S, dbg=False, split_waits=True):
    assert s % 128 == 0 and (s // 2 - 4) % 4 == 0
    nc = bass.Bass("TRN2", target_bir_lowering=False, debug=False,
                   num_devices=NCORES)

    npair = s // 2 + 2
    xd = nc.declare_dram_parameter("xpair", [npair, 3, 2, 2, HB], F16, isOutput=False)
    wgd = nc.declare_dram_parameter("Wg", [128, 4, 128], F16, isOutput=False)
    wo1d = nc.declare_dram_parameter("wout1", [128, 1], F16, isOutput=False)
    wo2d = nc.declare_dram_parameter("wout2", [128, 1], F16, isOutput=False)
    idd = nc.declare_dram_parameter("ident", [128, 128], F32, isOutput=False)
    outd = nc.declare_dram_parameter("out", [BL, s], F32, isOutput=True)
    # [pair, d, half, hb] == row-major [t, half, hb]
    oscr = nc.dram_tensor("oscr", [s // 2, 2, 2, HB], F32)

    with tile.TileContext(nc) as tc:
        with (
            tc.tile_pool(name="const", bufs=1) as cpool,
            tc.tile_pool(name="state", bufs=1) as spool,
            tc.tile_pool(name="psum", bufs=1, space="PSUM") as ppool,
        ):
            wg_t = cpool.tile([128, 4, 128], F16)
            wo1_t = cpool.tile([128, 1], F16)
            wo2_t = cpool.tile([128, 1], F16)
            id_t = cpool.tile([128, 128], F32)
            nc.sync.dma_start(wg_t[:], wgd[:])
            nc.sync.dma_start(wo1_t[:], wo1d[:])
            nc.sync.dma_start(wo2_t[:], wo2d[:])
            nc.sync.dma_start(id_t[:], idd[:])

            # sig/tct/c_t are double-buffered by wave parity so a wave's
            # consumers never WAR-block the next wave's producers.
            pkq = spool.tile([128, 4, 2, HB], F16, name="pkq")
            c_t = spool.tile([128, 2, 2, HB], F16, name="c_t")
            sig = spool.tile([128, 2, 2, 4 * HB], F16, name="sig")
            tct = spool.tile([128, 2, 2, HB], F16, name="tct")
            u3 = spool.tile([128, 2, HB], F16, name="u3")
            v_t = spool.tile([128, 2, HB], F16, name="v_t")
            hb1 = spool.tile([128, 2, 2, HB], F16, name="hb1")
            osb = spool.tile([1, 2, 2, 4 * HB], F32, name="osb")

            gps = ppool.tile([128, 2, 4 * HB], F32, name="gps")
            ops_ = ppool.tile([1, 2, 8 * HB], F32, name="ops")

            # ---- init ----
            nc.vector.memset(pkq[:], 0.0)
            nc.vector.memset(c_t[:], 0.0)
            nc.vector.memset(hb1[:], 0.0)

            def phase1(wm, h, do_out, do_wo2):
                """wm = wave index mod 8."""
                j = wm % 4
                p = pkq[:, j, h, :]
                if do_wo2:  # deferred wout2 for wave w-1
                    sl = (wm - 1) % 8
                    nc.tensor.matmul(
                        ops_[0:1, h, sl * HB : (sl + 1) * HB], wo2_t[:],
                        hb1[:, h, (wm - 1) % 2, :],
                        start=False, stop=True, skip_group_check=True,
                    )
                for g in range(4):
                    nc.tensor.matmul(
                        gps[:, h, g * HB : (g + 1) * HB],
                        wg_t[:, g, :], p, start=True, stop=True,
                    )
                if do_out:
                    nc.tensor.matmul(
                        ops_[0:1, h, wm * HB : (wm + 1) * HB], wo1_t[:], p,
                        start=True, stop=False, skip_group_check=True,
                    )
                pb = wm % 2
                nc.scalar.activation(sig[:, h, pb, :], gps[:, h, :], AF.Sigmoid)
                i_s = sig[:, h, pb, 0:HB]
                f_s = sig[:, h, pb, HB : 2 * HB]
                g_s = sig[:, h, pb, 2 * HB : 3 * HB]
                # u3 = (sigma(2g) - 0.5) * sigma(i)  == tanh(g)*sigma(i)/2
                nc.vector.scalar_tensor_tensor(
                    u3[:, h, :], g_s, -0.5, i_s, OP.add, OP.mult)
                nc.vector.tensor_tensor(
                    v_t[:, h, :], f_s, c_t[:, h, 1 - pb, :], OP.mult)
                nc.vector.scalar_tensor_tensor(
                    c_t[:, h, pb, :], u3[:, h, :], 2.0, v_t[:, h, :],
                    OP.mult, OP.add)

            def phase2(wm, h):
                j = wm % 4
                pb = wm % 2
                nc.scalar.activation(tct[:, h, pb, :], c_t[:, h, pb, :], AF.Tanh)
                nc.vector.tensor_tensor(
                    pkq[0:96, (j + 1) % 4, h, :],
                    sig[0:96, h, pb, 3 * HB : 4 * HB],
                    tct[0:96, h, pb, :], OP.mult,
                )
                nc.gpsimd.tensor_tensor(
                    hb1[96:128, h, wm % 2, :],
                    sig[96:128, h, pb, 3 * HB : 4 * HB],
                    tct[96:128, h, pb, :], OP.mult,
                )

            def evac4(wm, h, dst):
                """copy the older 4 ops slots to SBUF and DMA out.
                At wm==0: slots 4..7 (buf 1); at wm==4: slots 0..3 (buf 0)."""
                base, buf = (4, 1) if wm == 0 else (0, 0)
                nc.vector.tensor_copy(
                    osb[0:1, h, buf, :],
                    ops_[0:1, h, base * HB : (base + 4) * HB])
                nc.sync.dma_start(dst, osb[0:1, h, buf, :])

            def xload(pair, j0):
                nc.sync.dma_start(
                    pkq[96:99, j0 : j0 + 2, :, :],
                    xd[pair : pair + 1, :, :, :, :]
                    if isinstance(pair, int)
                    else xd[pair, :, :, :, :])

            def full_wave(wm, do_out, do_wo2, ed=None, peel23=False):
                for h in range(2):
                    phase1(wm, h, do_out, do_wo2)
                for h in range(2):
                    phase2(wm, h)
                for h in range(2):
                    if peel23:
                        # waves 2,3 (slots 2,3) -> rows 0,1
                        nc.vector.tensor_copy(
                            osb[0:1, h, 0, 0 : 2 * HB],
                            ops_[0:1, h, 2 * HB : 4 * HB])
                        nc.sync.dma_start(oscr[0:1, :, h, :],
                                          osb[0:1, h, 0, 0 : 2 * HB])
                    elif ed is not None:
                        evac4(wm, h, ed(h))

            # ---- prologue: waves 0,1 (skew priming, no out) ----
            xload(0, 0)
            xload(1, 2)
            full_wave(0, do_out=False, do_wo2=False)
            # zero junk written into hf1/hb0 rows of pkq slot 1 and cf1/cb0
            nc.vector.memset(pkq[32:64, 1, :, :], 0.0)
            nc.vector.memset(pkq[64:96, 1, :, :], 0.0)
            nc.vector.memset(c_t[32:64, :, 0, :], 0.0)
            nc.vector.memset(c_t[64:96, :, 0, :], 0.0)
            full_wave(1, do_out=False, do_wo2=False)
            nc.vector.memset(c_t[96:128, :, 1, :], 0.0)
            xload(2, 0)

            # ---- peel: waves 2..9 (pairs 1..4) ----
            for w in range(2, 10):
                wm = w % 8
                if w % 2 == 0 and w >= 4:
                    p = w // 2  # current pair: prefetch pair p+1
                    xload(p + 1, 0 if (p + 1) % 2 == 0 else 2)
                ed = None
                if w == 8:
                    # waves 4..7 (slots 4..7) -> rows 2..5 (pairs 1,2)
                    def ed(h):
                        return oscr[1:3, :, h, :]
                full_wave(wm, do_out=True, do_wo2=(w >= 3),
                          ed=ed, peel23=(w == 4))

            # ---- main loop: pairs 5 .. s//2, waves 10 .. s+1 ----
            with tc.For_i(5, s // 2 + 1, 4) as ip:
                for k in range(4):
                    # pair p = ip + k covers waves 2p, 2p+1
                    # (ip = 1 mod 4) -> wave mod 8 = 2+2k, 3+2k
                    xload(bass.ds(ip + k + 1, 1), 0 if k % 2 == 0 else 2)
                    for d in range(2):
                        wm = (2 + 2 * k + d) % 8
                        ed = None
                        if wm == 4:  # w = 2ip+2: slots 0..3 -> pairs ip-2, ip-1
                            def ed(h, _ip=ip):
                                return oscr[bass.ds(_ip - 2, 2), :, h, :]
                        elif wm == 0:  # w = 2ip+6: slots 4..7 -> pairs ip, ip+1
                            def ed(h, _ip=ip):
                                return oscr[bass.ds(_ip, 2), :, h, :]
                        full_wave(wm, do_out=True, do_wo2=True, ed=ed)

            # ---- epilogue: wo2 for wave s+1; evac waves s, s+1 (slots 0,1) --
            for h in range(2):
                # (s+1) % 8 == 1 for s % 8 == 0; hb1 ring (s+1) % 2 == 1
                nc.tensor.matmul(
                    ops_[0:1, h, HB : 2 * HB], wo2_t[:], hb1[:, h, 1, :],
                    start=False, stop=True, skip_group_check=True)
                nc.vector.tensor_copy(
                    osb[0:1, h, 0, 0 : 2 * HB], ops_[0:1, h, 0 : 2 * HB])
                nc.sync.dma_start(oscr[s // 2 - 1 : s // 2, :, h, :],
                                  osb[0:1, h, 0, 0 : 2 * HB])

            # ---- end phase: transpose oscr [t, half, b2] -> out [b, t] ----
            nchunk = s // 128
            outsb = spool.tile([128, 2, nchunk, 128], F32, name="outsb")
            stg = [spool.tile([128, 2, HB], F32, name=f"stg{j}") for j in range(2)]
            for ci in range(nchunk):
                st = stg[ci % 2]
                nc.sync.dma_start(st[:], oscr[ci * 64 : (ci + 1) * 64, :, :, :])
                for g in range(2):
                    tp = gps[:, g, 0:128]
                    nc.tensor.transpose(tp, st[:, g, :], id_t[:])
                    nc.vector.tensor_copy(outsb[:, g, ci, :], tp)
            nc.sync.dma_start(outd[0:HB, :], outsb[:, 0, :, :])
            nc.sync.dma_start(outd[HB : 2 * HB, :], outsb[:, 1, :, :])

    if split_waits:
        _split_excess_waits(nc)
    return nc


_NC_CACHE = {}


def _get_nc(s=S, dbg=False):
    key = (s, dbg)
    if key not in _NC_CACHE:
        _NC_CACHE[key] = build_nc(s, dbg)
    return _NC_CACHE[key]


def run(x, weights, s=S, dbg=False, trace=False):
    """x: [s, B] fp32 (already squeezed); weights: dict of reference arrays."""
    Wg, wout1, wout2 = build_weights(**weights)
    nc = _get_nc(s, dbg)
    ident = np.eye(128, dtype=np.float32)
    in_maps = []
    for c in range(NCORES):
        xs = np.ascontiguousarray(x[:, c * BL : (c + 1) * BL])
        in_maps.append(
            {"xpair": build_xpair(xs, s),
             "Wg": Wg, "wout1": wout1, "wout2": wout2, "ident": ident}
        )
    res = run_bass_kernel_spmd(nc, in_maps, list(range(NCORES)), trace=trace)
    out = np.concatenate([res.results[c]["out"] for c in range(NCORES)], axis=0)
    return out, res


def kernel(x, Wih_f0, Whh_f0, b_f0, Wih_f1, Whh_f1, b_f1,
           Wih_b0, Whh_b0, b_b0, Wih_b1, Whh_b1, b_b1, Wlin, blin, future):
    assert int(future) == 0, "kernel hardcodes future=0"
    x = np.asarray(x, np.float32)
    s, b, _ = x.shape
    assert (s, b) == (S, B)
    weights = dict(
        Wih_f0=np.asarray(Wih_f0, np.float32), Whh_f0=np.asarray(Whh_f0, np.float32),
        b_f0=np.asarray(b_f0, np.float32),
        Wih_f1=np.asarray(Wih_f1, np.float32), Whh_f1=np.asarray(Whh_f1, np.float32),
        b_f1=np.asarray(b_f1, np.float32),
        Wih_b0=np.asarray(Wih_b0, np.float32), Whh_b0=np.asarray(Whh_b0, np.float32),
        b_b0=np.asarray(b_b0, np.float32),
        Wih_b1=np.asarray(Wih_b1, np.float32), Whh_b1=np.asarray(Whh_b1, np.float32),
        b_b1=np.asarray(b_b1, np.float32),
        Wlin=np.asarray(Wlin, np.float32), blin=np.asarray(blin, np.float32),
    )
    out, _ = run(x[:, :, 0], weights, s=S)
    return out


# revision 15
# speedup vs baseline: 2.2626x; 1.0633x over previous
"""Bidirectional 2-layer LSTM (with replicated hf1-input bug) + per-step linear,
as a Trainium2 Bass/Tile kernel, data-parallel over batch across 8 NeuronCores.

v2: fp16 datapath + two phase-shifted half-batch pipelines per core.

Layout strategy (per core, B_loc=256 batch split into halves A/B of 128):
  - packed state tile pkq [128 rows, 4 slots, 2 halves, 128 batch] fp16:
      rows 0:32 hf0, 32:64 hf1, 64:96 hb0; row 96 = x[t], 97 = xb[t],
      98 = ones (bias enters via the ones row).
  - per half-wave: 4 fp16 matmuls (one per gate i,f,g,o), K=128 x M=128 x
    N=128, into a per-half PSUM bank; the g-gate weights are pre-scaled by 2
    so one merged Sigmoid yields sigma(2g), and tanh(g) = 2*sigma(2g)-1.
  - Scalar engine: one Sigmoid over [128, 512] + one Tanh over the fp16 cell
    state [128, 128] per half-wave.  This is the bottleneck engine; all other
    work overlaps under it because the two half-batch chains are interleaved
    (phase1 A, phase1 B, phase2 A, phase2 B) so every in-order engine queue
    always has ready work from the other half.
  - DVE: v = sig_f*c ; c = 2*u3 + v (scalar_tensor_tensor); h2a = sig_o *
    tanh(c) for cells f0/f1/b0 -> next pkq slot.  GpSimd: u3 =
    (sig_2g-0.5)*sig_i and h2b for cell b1 (feeds out-proj only).
  - Output projection: wout1 over packed (hf1 + bias, start) and wout2 over
    hb1 (stop) accumulate out[t] per half into an 8-slot PSUM ring; wout2 is
    emitted one wave late so the PE queue never waits on GpSimd.  Every 4
    waves the older 4 slots go PSUM->SBUF (DVE copy) -> DRAM scratch in
    [t, half, b] layout; PE-transposed to [b, t] at the end.
"""

import sys

sys.path.insert(0, "/opt/trn_rl_repo")

import numpy as np
import concourse.bass as bass
import concourse.tile as tile
import concourse.mybir as mybir
import bass_rust
from concourse.bass_utils import run_bass_kernel_spmd

S, B, H = 1024, 2048, 32
NCORES = 8
BL = B // NCORES  # 256 per-core batch
HB = BL // 2      # 128 half-batch

F32 = mybir.dt.float32
F16 = mybir.dt.float16
AF = mybir.ActivationFunctionType
OP = mybir.AluOpType

# cell order along M-columns / state partitions: [f0, f1, b0, b1]
CELL_COL = {"f0": 0, "f1": 32, "b0": 64, "b1": 96}
ROW_HF0, ROW_HF1, ROW_HB0 = 0, 32, 64
ROW_X, ROW_XB, ROW_ONES = 96, 97, 98


def _split_excess_waits(nc, max_waits=1):
    """walrus codegen in this toolchain supports only one sync-wait per
    instruction; split extras onto inserted wait-only drains."""
    n = 0
    for f in nc.m.functions:
        for bb in f.blocks:
            newl = []
            dirty = False
            for ins in bb.instructions:
                si = ins.sync_info
                waits = list(si.on_wait) if si is not None else []
                if len(waits) > max_waits:
                    dirty = True
                    k = len(waits) - max_waits
                    i = 0
                    while i < k:
                        chunk = waits[i : min(i + max_waits, k)]
                        d = mybir.InstDrain(name=f"zwsplit-{n}", is_reset_sema=False)
                        n += 1
                        d.engine = ins.engine
                        d.sync_info = bass_rust.SyncInfo(on_wait=chunk, on_update=[])
                        newl.append(d)
                        i += max_waits
                    si.on_wait = waits[k:]
                    ins.sync_info = si
                newl.append(ins)
            if dirty:
                bb.instructions = newl
    return n


def _gate_block(Wmat, gi):
    """rows of a torch 4H-row weight/bias for gate gi (torch order i,f,g,o)."""
    return Wmat[gi * H : (gi + 1) * H]


def build_weights(Wih_f0, Whh_f0, b_f0, Wih_f1, Whh_f1, b_f1,
                  Wih_b0, Whh_b0, b_b0, Wih_b1, Whh_b1, b_b1, Wlin, blin):
    """Pack per-gate stationary matrices Wg -> [K=128, gate, M=128] plus the
    two output-projection columns (all fp16)."""
    Wg = np.zeros((4, 128, 128), np.float32)
    for gi in range(4):
        sc = 2.0 if gi == 2 else 1.0  # tanh-gate pre-scale
        c = CELL_COL["f0"]  # inp = x, h = hf0
        Wg[gi, ROW_X, c : c + H] = _gate_block(Wih_f0, gi)[:, 0] * sc
        Wg[gi, ROW_ONES, c : c + H] = _gate_block(b_f0, gi) * sc
        Wg[gi, ROW_HF0 : ROW_HF0 + H, c : c + H] = _gate_block(Whh_f0, gi).T * sc
        c = CELL_COL["f1"]  # inp = hf0, h = hf1
        Wg[gi, ROW_ONES, c : c + H] = _gate_block(b_f1, gi) * sc
        Wg[gi, ROW_HF0 : ROW_HF0 + H, c : c + H] = _gate_block(Wih_f1, gi).T * sc
        Wg[gi, ROW_HF1 : ROW_HF1 + H, c : c + H] = _gate_block(Whh_f1, gi).T * sc
        c = CELL_COL["b0"]  # inp = xb, h = hb0
        Wg[gi, ROW_XB, c : c + H] = _gate_block(Wih_b0, gi)[:, 0] * sc
        Wg[gi, ROW_ONES, c : c + H] = _gate_block(b_b0, gi) * sc
        Wg[gi, ROW_HB0 : ROW_HB0 + H, c : c + H] = _gate_block(Whh_b0, gi).T * sc
        c = CELL_COL["b1"]  # inp = hb0, h-arg = hf1 (replicated bug)
        Wg[gi, ROW_ONES, c : c + H] = _gate_block(b_b1, gi) * sc
        Wg[gi, ROW_HB0 : ROW_HB0 + H, c : c + H] = _gate_block(Wih_b1, gi).T * sc
        Wg[gi, ROW_HF1 : ROW_HF1 + H, c : c + H] = _gate_block(Whh_b1, gi).T * sc

    wout1 = np.zeros((128, 1), np.float32)
    wout1[ROW_ONES, 0] = blin[0]
    wout1[ROW_HF1 : ROW_HF1 + H, 0] = Wlin[0, 0:H]
    wout2 = np.zeros((128, 1), np.float32)
    wout2[96:128, 0] = Wlin[0, H : 2 * H]
    return (np.ascontiguousarray(Wg.transpose(1, 0, 2)).astype(np.float16),
            wout1.astype(np.float16), wout2.astype(np.float16))


def build_xpair(x_shard, s):
    """Interleave per-wave x rows: xp[p, r, d, h, :] is the row for packed
    partition 96+r (0 = x, 1 = xb, 2 = ones) of wave w = 2p + d, half h."""
    bl = x_shard.shape[1]
    hb = bl // 2
    npair = s // 2 + 2  # one pad pair for the loop's depth-1 prefetch
    xp = np.zeros((npair, 3, 2, 2, hb), np.float16)
    xp[:, 2] = 1.0
    x16 = x_shard.astype(np.float16)
    xp[0 : s // 2, 0] = x16.reshape(s // 2, 2, 2, hb)
    # xb rows: wave w in 1..s+1 reads x[(s + 1 - w) % s]
    w = np.arange(1, s + 2)
    xb = x16[(s + 1 - w) % s].reshape(-1, 2, hb)  # [s+1, 2, hb]
    xbp = np.zeros((npair * 2, 2, hb), np.float16)
    xbp[1 : s + 2] = xb
    xp[:, 1] = xbp.reshape(npair, 2, 2, hb)
    return xp


def build_nc(s=S, dbg=False, split_waits=True):
    assert s % 128 == 0 and (s // 2 - 4) % 4 == 0
    nc = bass.Bass("TRN2", target_bir_lowering=False, debug=False,
                   num_devices=NCORES)

    npair = s // 2 + 2
    xd = nc.declare_dram_parameter("xpair", [npair, 3, 2, 2, HB], F16, isOutput=False)
    wgd = nc.declare_dram_parameter("Wg", [128, 4, 128], F16, isOutput=False)
    wo1d = nc.declare_dram_parameter("wout1", [128, 1], F16, isOutput=False)
    wo2d = nc.declare_dram_parameter("wout2", [128, 1], F16, isOutput=False)
    idd = nc.declare_dram_parameter("ident", [128, 128], F32, isOutput=False)
    outd = nc.declare_dram_parameter("out", [BL, s], F32, isOutput=True)
    # [pair, d, half, hb] == row-major [t, half, hb]
    oscr = nc.dram_tensor("oscr", [s // 2, 2, 2, HB], F32)

    with tile.TileContext(nc) as tc:
        with (
            tc.tile_pool(name="const", bufs=1) as cpool,
            tc.tile_pool(name="state", bufs=1) as spool,
            tc.tile_pool(name="psum", bufs=1, space="PSUM") as ppool,
        ):
            wg_t = cpool.tile([128, 4, 128], F16)
            wo1_t = cpool.tile([128, 1], F16)
            wo2_t = cpool.tile([128, 1], F16)
            id_t = cpool.tile([128, 128], F32)
            nc.sync.dma_start(wg_t[:], wgd[:])
            nc.sync.dma_start(wo1_t[:], wo1d[:])
            nc.sync.dma_start(wo2_t[:], wo2d[:])
            nc.sync.dma_start(id_t[:], idd[:])

            # sig/tct/c_t are double-buffered by wave parity so a wave's
            # consumers never WAR-block the next wave's producers.  hb1 is an
            # 8-deep ring (indexed by wave mod 8) so the batched wout2 matmul
            # reads 4 stable slots with no WAR against h2b.
            pkq = spool.tile([128, 4, 2, HB], F16, name="pkq")
            c_t = spool.tile([128, 2, 2, HB], F16, name="c_t")
            sig = spool.tile([128, 2, 2, 4 * HB], F16, name="sig")
            tct = spool.tile([128, 2, 2, HB], F16, name="tct")
            u3 = spool.tile([128, 2, HB], F16, name="u3")
            v_t = spool.tile([128, 2, HB], F16, name="v_t")
            hb1 = spool.tile([128, 2, 8, HB], F16, name="hb1")
            osb = spool.tile([1, 2, 2, 4 * HB], F32, name="osb")

            gps = ppool.tile([128, 2, 4 * HB], F32, name="gps")
            # out accumulator: [half, chunk-ring(2), 4 waves x 128 batch]
            ops_ = ppool.tile([1, 2, 2, 4 * HB], F32, name="ops")

            # ---- init ----
            nc.vector.memset(pkq[:], 0.0)
            nc.vector.memset(c_t[:], 0.0)
            nc.vector.memset(hb1[:, :, 0:4, :], 0.0)
            nc.vector.memset(hb1[:, :, 4:8, :], 0.0)

            def wo1_batch(h, cbuf):
                # out[t] partial for 4 waves at once: wout1 over pkq slots 0..3
                nc.tensor.matmul(
                    ops_[0:1, h, cbuf, :], wo1_t[:], pkq[:, 0:4, h, :],
                    start=True, stop=False, skip_group_check=True)

            def wo2_batch(h, cbuf, rbase):
                nc.tensor.matmul(
                    ops_[0:1, h, cbuf, :], wo2_t[:],
                    hb1[:, h, rbase : rbase + 4, :],
                    start=False, stop=True, skip_group_check=True)

            def phase1(wm, h, wo1_c=None, wo2_c=None):
                """wm = wave index mod 8."""
                j = wm % 4
                pb = wm % 2
                p = pkq[:, j, h, :]
                if wo2_c is not None:
                    wo2_batch(h, wo2_c[0], wo2_c[1])
                for g in range(4):
                    nc.tensor.matmul(
                        gps[:, h, g * HB : (g + 1) * HB],
                        wg_t[:, g, :], p, start=True, stop=True,
                    )
                if wo1_c is not None:
                    wo1_batch(h, wo1_c)
                nc.scalar.activation(sig[:, h, pb, :], gps[:, h, :], AF.Sigmoid)
                i_s = sig[:, h, pb, 0:HB]
                f_s = sig[:, h, pb, HB : 2 * HB]
                g_s = sig[:, h, pb, 2 * HB : 3 * HB]
                # u3 = (sigma(2g) - 0.5) * sigma(i)  == tanh(g)*sigma(i)/2
                nc.vector.scalar_tensor_tensor(
                    u3[:, h, :], g_s, -0.5, i_s, OP.add, OP.mult)
                nc.vector.tensor_tensor(
                    v_t[:, h, :], f_s, c_t[:, h, 1 - pb, :], OP.mult)
                nc.vector.scalar_tensor_tensor(
                    c_t[:, h, pb, :], u3[:, h, :], 2.0, v_t[:, h, :],
                    OP.mult, OP.add)

            def phase2(wm, h):
                j = wm % 4
                pb = wm % 2
                nc.scalar.activation(tct[:, h, pb, :], c_t[:, h, pb, :], AF.Tanh)
                nc.vector.tensor_tensor(
                    pkq[0:96, (j + 1) % 4, h, :],
                    sig[0:96, h, pb, 3 * HB : 4 * HB],
                    tct[0:96, h, pb, :], OP.mult,
                )
                nc.gpsimd.tensor_tensor(
                    hb1[96:128, h, wm, :],
                    sig[96:128, h, pb, 3 * HB : 4 * HB],
                    tct[96:128, h, pb, :], OP.mult,
                )

            def evac(h, cbuf, dst, half_only=False):
                if half_only:
                    nc.vector.tensor_copy(
                        osb[0:1, h, cbuf, 0 : 2 * HB],
                        ops_[0:1, h, cbuf, 2 * HB : 4 * HB])
                    nc.sync.dma_start(dst, osb[0:1, h, cbuf, 0 : 2 * HB])
                else:
                    nc.vector.tensor_copy(
                        osb[0:1, h, cbuf, :], ops_[0:1, h, cbuf, :])
                    nc.sync.dma_start(dst, osb[0:1, h, cbuf, :])

            def xload(pair, j0):
                nc.sync.dma_start(
                    pkq[96:99, j0 : j0 + 2, :, :],
                    xd[pair : pair + 1, :, :, :, :]
                    if isinstance(pair, int)
                    else xd[pair, :, :, :, :])

            def full_wave(wm, wo1_c=None, wo2_c=None, ev=None):
                for h in range(2):
                    phase1(wm, h, wo1_c, wo2_c)
                for h in range(2):
                    phase2(wm, h)
                if ev is not None:
                    cbuf, dstf, half_only = ev
                    for h in range(2):
                        evac(h, cbuf, dstf(h), half_only)

            # ---- prologue: waves 0,1 (skew priming, no out) ----
            xload(0, 0)
            xload(1, 2)
            full_wave(0)
            # zero junk written into hf1/hb0 rows of pkq slot 1 and cf1/cb0
            nc.vector.memset(pkq[32:64, 1, :, :], 0.0)
            nc.vector.memset(pkq[64:96, 1, :, :], 0.0)
            nc.vector.memset(c_t[32:64, :, 0, :], 0.0)
            nc.vector.memset(c_t[64:96, :, 0, :], 0.0)
            full_wave(1)
            nc.vector.memset(c_t[96:128, :, 1, :], 0.0)
            xload(2, 0)

            # ---- peel: waves 2..9 (pairs 1..4) ----
            # chunk cbuf0 = waves 0..3 (only waves 2,3 kept -> rows 0,1);
            # chunk cbuf1 = waves 4..7 (evac'd by the loop's first wave).
            for w in range(2, 10):
                wm = w % 8
                if w % 2 == 0 and w >= 4:
                    p = w // 2  # current pair: prefetch pair p+1
                    xload(p + 1, 0 if (p + 1) % 2 == 0 else 2)
                kw = {}
                if w == 3:
                    kw["wo1_c"] = 0
                elif w == 5:
                    kw["wo2_c"] = (0, 0)
                elif w == 6:
                    kw["ev"] = (0, lambda h: oscr[0:1, :, h, :], True)
                elif w == 7:
                    kw["wo1_c"] = 1
                elif w == 9:
                    kw["wo2_c"] = (1, 4)
                full_wave(wm, **kw)

            # ---- main loop: pairs 5 .. s//2, waves 10 .. s+1 ----
            with tc.For_i(5, s // 2 + 1, 4) as ip:
                for k in range(4):
                    # pair p = ip + k covers waves 2p, 2p+1
                    # (ip = 1 mod 4) -> wave mod 8 = 2+2k, 3+2k
                    xload(bass.ds(ip + k + 1, 1), 0 if k % 2 == 0 else 2)
                    for d in range(2):
                        wm = (2 + 2 * k + d) % 8
                        kw = {}
                        if wm == 2:   # evac prev body cbuf1: pairs ip-4,ip-3
                            kw["ev"] = (1, lambda h, _ip=ip:
                                        oscr[bass.ds(_ip - 4, 2), :, h, :],
                                        False)
                        elif wm == 3:
                            kw["wo1_c"] = 0
                        elif wm == 5:
                            kw["wo2_c"] = (0, 0)
                        elif wm == 6:  # evac cbuf0: pairs ip-2, ip-1
                            kw["ev"] = (0, lambda h, _ip=ip:
                                        oscr[bass.ds(_ip - 2, 2), :, h, :],
                                        False)
                        elif wm == 7:
                            kw["wo1_c"] = 1
                        elif wm == 1:
                            kw["wo2_c"] = (1, 4)
                        full_wave(wm, **kw)

            # ---- epilogue ----
            for h in range(2):
                # evac last body's cbuf1 (waves s-4..s-1 -> pairs 509,510)
                evac(h, 1, oscr[s // 2 - 3 : s // 2 - 1, :, h, :])
                # waves s, s+1 (pkq slots 0,1; hb1 rings 0,1) -> rows s-2, s-1
                nc.tensor.matmul(
                    ops_[0:1, h, 0, 0 : 2 * HB], wo1_t[:], pkq[:, 0:2, h, :],
                    start=True, stop=False, skip_group_check=True)
                nc.tensor.matmul(
                    ops_[0:1, h, 0, 0 : 2 * HB], wo2_t[:], hb1[:, h, 0:2, :],
                    start=False, stop=True, skip_group_check=True)
                nc.vector.tensor_copy(
                    osb[0:1, h, 0, 0 : 2 * HB], ops_[0:1, h, 0, 0 : 2 * HB])
                nc.sync.dma_start(oscr[s // 2 - 1 : s // 2, :, h, :],
                                  osb[0:1, h, 0, 0 : 2 * HB])

            # ---- end phase: transpose oscr [t, half, b2] -> out [b, t] ----
            nchunk = s // 128
            outsb = spool.tile([128, 2, nchunk, 128], F32, name="outsb")
            stg = [spool.tile([128, 2, HB], F32, name=f"stg{j}") for j in range(2)]
            for ci in range(nchunk):
                st = stg[ci % 2]
                nc.sync.dma_start(st[:], oscr[ci * 64 : (ci + 1) * 64, :, :, :])
                for g in range(2):
                    tp = gps[:, g, 0:128]
                    nc.tensor.transpose(tp, st[:, g, :], id_t[:])
                    nc.vector.tensor_copy(outsb[:, g, ci, :], tp)
            nc.sync.dma_start(outd[0:HB, :], outsb[:, 0, :, :])
            nc.sync.dma_start(outd[HB : 2 * HB, :], outsb[:, 1, :, :])

    if split_waits:
        _split_excess_waits(nc)
    return nc


_NC_CACHE = {}


def _get_nc(s=S, dbg=False):
    key = (s, dbg)
    if key not in _NC_CACHE:
        _NC_CACHE[key] = build_nc(s, dbg)
    return _NC_CACHE[key]


def run(x, weights, s=S, dbg=False, trace=False):
    """x: [s, B] fp32 (already squeezed); weights: dict of reference arrays."""
    Wg, wout1, wout2 = build_weights(**weights)
    nc = _get_nc(s, dbg)
    ident = np.eye(128, dtype=np.float32)
    in_maps = []
    for c in range(NCORES):
        xs = np.ascontiguousarray(x[:, c * BL : (c + 1) * BL])
        in_maps.append(
            {"xpair": build_xpair(xs, s),
             "Wg": Wg, "wout1": wout1, "wout2": wout2, "ident": ident}
        )
    res = run_bass_kernel_spmd(nc, in_maps, list(range(NCORES)), trace=trace)
    out = np.concatenate([res.results[c]["out"] for c in range(NCORES)], axis=0)
    return out, res


def kernel(x, Wih_f0, Whh_f0, b_f0, Wih_f1, Whh_f1, b_f1,
           Wih_b0, Whh_b0, b_b0, Wih_b1, Whh_b1, b_b1, Wlin, blin, future):
    assert int(future) == 0, "kernel hardcodes future=0"
    x = np.asarray(x, np.float32)
    s, b, _ = x.shape
    assert (s, b) == (S, B)
    weights = dict(
        Wih_f0=np.asarray(Wih_f0, np.float32), Whh_f0=np.asarray(Whh_f0, np.float32),
        b_f0=np.asarray(b_f0, np.float32),
        Wih_f1=np.asarray(Wih_f1, np.float32), Whh_f1=np.asarray(Whh_f1, np.float32),
        b_f1=np.asarray(b_f1, np.float32),
        Wih_b0=np.asarray(Wih_b0, np.float32), Whh_b0=np.asarray(Whh_b0, np.float32),
        b_b0=np.asarray(b_b0, np.float32),
        Wih_b1=np.asarray(Wih_b1, np.float32), Whh_b1=np.asarray(Whh_b1, np.float32),
        b_b1=np.asarray(b_b1, np.float32),
        Wlin=np.asarray(Wlin, np.float32), blin=np.asarray(blin, np.float32),
    )
    out, _ = run(x[:, :, 0], weights, s=S)
    return out


# revision 16
# speedup vs baseline: 2.2631x; 1.0003x over previous
"""Bidirectional 2-layer LSTM (with replicated hf1-input bug) + per-step linear,
as a Trainium2 Bass/Tile kernel, data-parallel over batch across 8 NeuronCores.

v2: fp16 datapath + two phase-shifted half-batch pipelines per core.

Layout strategy (per core, B_loc=256 batch split into halves A/B of 128):
  - packed state tile pkq [128 rows, 4 slots, 2 halves, 128 batch] fp16:
      rows 0:32 hf0, 32:64 hf1, 64:96 hb0; row 96 = x[t], 97 = xb[t],
      98 = ones (bias enters via the ones row).
  - per half-wave: 4 fp16 matmuls (one per gate i,f,g,o), K=128 x M=128 x
    N=128, into a per-half PSUM bank; the g-gate weights are pre-scaled by 2
    so one merged Sigmoid yields sigma(2g), and tanh(g) = 2*sigma(2g)-1.
  - Scalar engine: one Sigmoid over [128, 512] + one Tanh over the fp16 cell
    state [128, 128] per half-wave.  This is the bottleneck engine; all other
    work overlaps under it because the two half-batch chains are interleaved
    (phase1 A, phase1 B, phase2 A, phase2 B) so every in-order engine queue
    always has ready work from the other half.
  - DVE: v = sig_f*c ; c = 2*u3 + v (scalar_tensor_tensor); h2a = sig_o *
    tanh(c) for cells f0/f1/b0 -> next pkq slot.  GpSimd: u3 =
    (sig_2g-0.5)*sig_i and h2b for cell b1 (feeds out-proj only).
  - Output projection: wout1 over packed (hf1 + bias, start) and wout2 over
    hb1 (stop) accumulate out[t] per half into an 8-slot PSUM ring; wout2 is
    emitted one wave late so the PE queue never waits on GpSimd.  Every 4
    waves the older 4 slots go PSUM->SBUF (DVE copy) -> DRAM scratch in
    [t, half, b] layout; PE-transposed to [b, t] at the end.
"""

import sys

sys.path.insert(0, "/opt/trn_rl_repo")

import numpy as np
import concourse.bass as bass
import concourse.tile as tile
import concourse.mybir as mybir
import bass_rust
from concourse.bass_utils import run_bass_kernel_spmd

S, B, H = 1024, 2048, 32
NCORES = 8
BL = B // NCORES  # 256 per-core batch
HB = BL // 2      # 128 half-batch

F32 = mybir.dt.float32
F16 = mybir.dt.float16
AF = mybir.ActivationFunctionType
OP = mybir.AluOpType

# cell order along M-columns / state partitions: [f0, f1, b0, b1]
CELL_COL = {"f0": 0, "f1": 32, "b0": 64, "b1": 96}
ROW_HF0, ROW_HF1, ROW_HB0 = 0, 32, 64
ROW_X, ROW_XB, ROW_ONES = 96, 97, 98


def _split_excess_waits(nc, max_waits=1):
    """walrus codegen in this toolchain supports only one sync-wait per
    instruction; split extras onto inserted wait-only drains."""
    n = 0
    for f in nc.m.functions:
        for bb in f.blocks:
            newl = []
            dirty = False
            for ins in bb.instructions:
                si = ins.sync_info
                waits = list(si.on_wait) if si is not None else []
                if len(waits) > max_waits:
                    dirty = True
                    k = len(waits) - max_waits
                    i = 0
                    while i < k:
                        chunk = waits[i : min(i + max_waits, k)]
                        d = mybir.InstDrain(name=f"zwsplit-{n}", is_reset_sema=False)
                        n += 1
                        d.engine = ins.engine
                        d.sync_info = bass_rust.SyncInfo(on_wait=chunk, on_update=[])
                        newl.append(d)
                        i += max_waits
                    si.on_wait = waits[k:]
                    ins.sync_info = si
                newl.append(ins)
            if dirty:
                bb.instructions = newl
    return n


def _gate_block(Wmat, gi):
    """rows of a torch 4H-row weight/bias for gate gi (torch order i,f,g,o)."""
    return Wmat[gi * H : (gi + 1) * H]


def build_weights(Wih_f0, Whh_f0, b_f0, Wih_f1, Whh_f1, b_f1,
                  Wih_b0, Whh_b0, b_b0, Wih_b1, Whh_b1, b_b1, Wlin, blin):
    """Pack per-gate stationary matrices Wg -> [K=128, gate, M=128] plus the
    two output-projection columns (all fp16)."""
    Wg = np.zeros((4, 128, 128), np.float32)
    for gi in range(4):
        sc = 2.0 if gi == 2 else 1.0  # tanh-gate pre-scale
        c = CELL_COL["f0"]  # inp = x, h = hf0
        Wg[gi, ROW_X, c : c + H] = _gate_block(Wih_f0, gi)[:, 0] * sc
        Wg[gi, ROW_ONES, c : c + H] = _gate_block(b_f0, gi) * sc
        Wg[gi, ROW_HF0 : ROW_HF0 + H, c : c + H] = _gate_block(Whh_f0, gi).T * sc
        c = CELL_COL["f1"]  # inp = hf0, h = hf1
        Wg[gi, ROW_ONES, c : c + H] = _gate_block(b_f1, gi) * sc
        Wg[gi, ROW_HF0 : ROW_HF0 + H, c : c + H] = _gate_block(Wih_f1, gi).T * sc
        Wg[gi, ROW_HF1 : ROW_HF1 + H, c : c + H] = _gate_block(Whh_f1, gi).T * sc
        c = CELL_COL["b0"]  # inp = xb, h = hb0
        Wg[gi, ROW_XB, c : c + H] = _gate_block(Wih_b0, gi)[:, 0] * sc
        Wg[gi, ROW_ONES, c : c + H] = _gate_block(b_b0, gi) * sc
        Wg[gi, ROW_HB0 : ROW_HB0 + H, c : c + H] = _gate_block(Whh_b0, gi).T * sc
        c = CELL_COL["b1"]  # inp = hb0, h-arg = hf1 (replicated bug)
        Wg[gi, ROW_ONES, c : c + H] = _gate_block(b_b1, gi) * sc
        Wg[gi, ROW_HB0 : ROW_HB0 + H, c : c + H] = _gate_block(Wih_b1, gi).T * sc
        Wg[gi, ROW_HF1 : ROW_HF1 + H, c : c + H] = _gate_block(Whh_b1, gi).T * sc

    wout1 = np.zeros((128, 1), np.float32)
    wout1[ROW_ONES, 0] = blin[0]
    wout1[ROW_HF1 : ROW_HF1 + H, 0] = Wlin[0, 0:H]
    wout2 = np.zeros((128, 1), np.float32)
    wout2[96:128, 0] = Wlin[0, H : 2 * H]
    return (np.ascontiguousarray(Wg.transpose(1, 0, 2)).astype(np.float16),
            wout1.astype(np.float16), wout2.astype(np.float16))


def build_xpair(x_shard, s):
    """Interleave per-wave x rows: xp[p, r, d, h, :] is the row for packed
    partition 96+r (0 = x, 1 = xb, 2 = ones) of wave w = 2p + d, half h."""
    bl = x_shard.shape[1]
    hb = bl // 2
    npair = s // 2 + 2  # one pad pair for the loop's depth-1 prefetch
    xp = np.zeros((npair, 3, 2, 2, hb), np.float16)
    xp[:, 2] = 1.0
    x16 = x_shard.astype(np.float16)
    xp[0 : s // 2, 0] = x16.reshape(s // 2, 2, 2, hb)
    # xb rows: wave w in 1..s+1 reads x[(s + 1 - w) % s]
    w = np.arange(1, s + 2)
    xb = x16[(s + 1 - w) % s].reshape(-1, 2, hb)  # [s+1, 2, hb]
    xbp = np.zeros((npair * 2, 2, hb), np.float16)
    xbp[1 : s + 2] = xb
    xp[:, 1] = xbp.reshape(npair, 2, 2, hb)
    return xp


def build_nc(s=S, dbg=False, split_waits=True):
    assert s % 128 == 0 and (s // 2 - 4) % 4 == 0
    nc = bass.Bass("TRN2", target_bir_lowering=False, debug=False,
                   num_devices=NCORES)

    npair = s // 2 + 2
    xd = nc.declare_dram_parameter("xpair", [npair, 3, 2, 2, HB], F16, isOutput=False)
    wgd = nc.declare_dram_parameter("Wg", [128, 4, 128], F16, isOutput=False)
    wo1d = nc.declare_dram_parameter("wout1", [128, 1], F16, isOutput=False)
    wo2d = nc.declare_dram_parameter("wout2", [128, 1], F16, isOutput=False)
    idd = nc.declare_dram_parameter("ident", [128, 128], F32, isOutput=False)
    outd = nc.declare_dram_parameter("out", [BL, s], F32, isOutput=True)
    # [pair, d, half, hb] == row-major [t, half, hb]
    oscr = nc.dram_tensor("oscr", [s // 2, 2, 2, HB], F32)

    with tile.TileContext(nc) as tc:
        with (
            tc.tile_pool(name="const", bufs=1) as cpool,
            tc.tile_pool(name="state", bufs=1) as spool,
            tc.tile_pool(name="psum", bufs=1, space="PSUM") as ppool,
        ):
            wg_t = cpool.tile([128, 4, 128], F16)
            wo1_t = cpool.tile([128, 1], F16)
            wo2_t = cpool.tile([128, 1], F16)
            id_t = cpool.tile([128, 128], F32)
            nc.sync.dma_start(wg_t[:], wgd[:])
            nc.sync.dma_start(wo1_t[:], wo1d[:])
            nc.sync.dma_start(wo2_t[:], wo2d[:])
            nc.sync.dma_start(id_t[:], idd[:])

            # sig/tct/c_t are double-buffered by wave parity so a wave's
            # consumers never WAR-block the next wave's producers.  hb1 is an
            # 8-deep ring (indexed by wave mod 8) so the batched wout2 matmul
            # reads 4 stable slots with no WAR against h2b.
            pkq = spool.tile([128, 4, 2, HB], F16, name="pkq")
            c_t = spool.tile([128, 2, 2, HB], F16, name="c_t")
            sig = spool.tile([128, 2, 2, 4 * HB], F16, name="sig")
            tct = spool.tile([128, 2, 2, HB], F16, name="tct")
            u3 = spool.tile([128, 2, HB], F16, name="u3")
            v_t = spool.tile([128, 2, HB], F16, name="v_t")
            hb1 = spool.tile([128, 2, 8, HB], F16, name="hb1")
            osb = spool.tile([1, 2, 2, 4 * HB], F32, name="osb")

            gps = ppool.tile([128, 2, 4 * HB], F32, name="gps")
            # out accumulator: [half, chunk-ring(2), 4 waves x 128 batch]
            ops_ = ppool.tile([1, 2, 2, 4 * HB], F32, name="ops")

            # ---- init ----
            nc.vector.memset(pkq[:], 0.0)
            nc.vector.memset(c_t[:], 0.0)
            nc.vector.memset(hb1[:, :, 0:4, :], 0.0)
            nc.vector.memset(hb1[:, :, 4:8, :], 0.0)

            def wo1_batch(h, cbuf):
                # out[t] partial for 4 waves at once: wout1 over pkq slots 0..3
                nc.tensor.matmul(
                    ops_[0:1, h, cbuf, :], wo1_t[:], pkq[:, 0:4, h, :],
                    start=True, stop=False, skip_group_check=True)

            def wo2_batch(h, cbuf, rbase):
                nc.tensor.matmul(
                    ops_[0:1, h, cbuf, :], wo2_t[:],
                    hb1[:, h, rbase : rbase + 4, :],
                    start=False, stop=True, skip_group_check=True)

            def phase1(wm, h, wo1_c=None, wo2_c=None):
                """wm = wave index mod 8."""
                j = wm % 4
                pb = wm % 2
                p = pkq[:, j, h, :]
                if wo2_c is not None:
                    wo2_batch(h, wo2_c[0], wo2_c[1])
                for g in range(4):
                    nc.tensor.matmul(
                        gps[:, h, g * HB : (g + 1) * HB],
                        wg_t[:, g, :], p, start=True, stop=True,
                    )
                if wo1_c is not None:
                    wo1_batch(h, wo1_c)
                nc.scalar.activation(sig[:, h, pb, :], gps[:, h, :], AF.Sigmoid)
                i_s = sig[:, h, pb, 0:HB]
                f_s = sig[:, h, pb, HB : 2 * HB]
                g_s = sig[:, h, pb, 2 * HB : 3 * HB]
                # u3 = (sigma(2g) - 0.5) * sigma(i)  == tanh(g)*sigma(i)/2
                nc.vector.scalar_tensor_tensor(
                    u3[:, h, :], g_s, -0.5, i_s, OP.add, OP.mult)
                nc.vector.tensor_tensor(
                    v_t[:, h, :], f_s, c_t[:, h, 1 - pb, :], OP.mult)
                nc.vector.scalar_tensor_tensor(
                    c_t[:, h, pb, :], u3[:, h, :], 2.0, v_t[:, h, :],
                    OP.mult, OP.add)

            def phase2(wm, h):
                j = wm % 4
                pb = wm % 2
                nc.scalar.activation(tct[:, h, pb, :], c_t[:, h, pb, :], AF.Tanh)
                nc.vector.tensor_tensor(
                    pkq[0:96, (j + 1) % 4, h, :],
                    sig[0:96, h, pb, 3 * HB : 4 * HB],
                    tct[0:96, h, pb, :], OP.mult,
                )
                nc.gpsimd.tensor_tensor(
                    hb1[96:128, h, wm, :],
                    sig[96:128, h, pb, 3 * HB : 4 * HB],
                    tct[96:128, h, pb, :], OP.mult,
                )

            def evac(h, cbuf, dst, half_only=False):
                if half_only:
                    nc.vector.tensor_copy(
                        osb[0:1, h, cbuf, 0 : 2 * HB],
                        ops_[0:1, h, cbuf, 2 * HB : 4 * HB])
                    nc.sync.dma_start(dst, osb[0:1, h, cbuf, 0 : 2 * HB])
                else:
                    nc.vector.tensor_copy(
                        osb[0:1, h, cbuf, :], ops_[0:1, h, cbuf, :])
                    nc.sync.dma_start(dst, osb[0:1, h, cbuf, :])

            def xload(pair, j0):
                nc.sync.dma_start(
                    pkq[96:99, j0 : j0 + 2, :, :],
                    xd[pair : pair + 1, :, :, :, :]
                    if isinstance(pair, int)
                    else xd[pair, :, :, :, :])

            def full_wave(wm, wo1_c=None, wo2_c=None, ev=None):
                for h in range(2):
                    phase1(wm, h, wo1_c, wo2_c)
                # evacs sit here: the DVE idles waiting for tanh anyway
                if ev is not None:
                    cbuf, dstf, half_only = ev
                    for h in range(2):
                        evac(h, cbuf, dstf(h), half_only)
                for h in range(2):
                    phase2(wm, h)

            # ---- prologue: waves 0,1 (skew priming, no out) ----
            xload(0, 0)
            xload(1, 2)
            full_wave(0)
            # zero junk written into hf1/hb0 rows of pkq slot 1 and cf1/cb0
            nc.vector.memset(pkq[32:64, 1, :, :], 0.0)
            nc.vector.memset(pkq[64:96, 1, :, :], 0.0)
            nc.vector.memset(c_t[32:64, :, 0, :], 0.0)
            nc.vector.memset(c_t[64:96, :, 0, :], 0.0)
            full_wave(1)
            nc.vector.memset(c_t[96:128, :, 1, :], 0.0)
            xload(2, 0)

            # ---- peel: waves 2..9 (pairs 1..4) ----
            # chunk cbuf0 = waves 0..3 (only waves 2,3 kept -> rows 0,1);
            # chunk cbuf1 = waves 4..7 (evac'd by the loop's first wave).
            for w in range(2, 10):
                wm = w % 8
                if w % 2 == 0 and w >= 4:
                    p = w // 2  # current pair: prefetch pair p+1
                    xload(p + 1, 0 if (p + 1) % 2 == 0 else 2)
                kw = {}
                if w == 3:
                    kw["wo1_c"] = 0
                elif w == 5:
                    kw["wo2_c"] = (0, 0)
                elif w == 6:
                    kw["ev"] = (0, lambda h: oscr[0:1, :, h, :], True)
                elif w == 7:
                    kw["wo1_c"] = 1
                elif w == 9:
                    kw["wo2_c"] = (1, 4)
                full_wave(wm, **kw)

            # ---- main loop: pairs 5 .. s//2, waves 10 .. s+1 ----
            with tc.For_i(5, s // 2 + 1, 4) as ip:
                for k in range(4):
                    # pair p = ip + k covers waves 2p, 2p+1
                    # (ip = 1 mod 4) -> wave mod 8 = 2+2k, 3+2k
                    xload(bass.ds(ip + k + 1, 1), 0 if k % 2 == 0 else 2)
                    for d in range(2):
                        wm = (2 + 2 * k + d) % 8
                        kw = {}
                        if wm == 2:   # evac prev body cbuf1: pairs ip-4,ip-3
                            kw["ev"] = (1, lambda h, _ip=ip:
                                        oscr[bass.ds(_ip - 4, 2), :, h, :],
                                        False)
                        elif wm == 3:
                            kw["wo1_c"] = 0
                        elif wm == 5:
                            kw["wo2_c"] = (0, 0)
                        elif wm == 6:  # evac cbuf0: pairs ip-2, ip-1
                            kw["ev"] = (0, lambda h, _ip=ip:
                                        oscr[bass.ds(_ip - 2, 2), :, h, :],
                                        False)
                        elif wm == 7:
                            kw["wo1_c"] = 1
                        elif wm == 1:
                            kw["wo2_c"] = (1, 4)
                        full_wave(wm, **kw)

            # ---- epilogue ----
            for h in range(2):
                # evac last body's cbuf1 (waves s-4..s-1 -> pairs 509,510)
                evac(h, 1, oscr[s // 2 - 3 : s // 2 - 1, :, h, :])
                # waves s, s+1 (pkq slots 0,1; hb1 rings 0,1) -> rows s-2, s-1
                nc.tensor.matmul(
                    ops_[0:1, h, 0, 0 : 2 * HB], wo1_t[:], pkq[:, 0:2, h, :],
                    start=True, stop=False, skip_group_check=True)
                nc.tensor.matmul(
                    ops_[0:1, h, 0, 0 : 2 * HB], wo2_t[:], hb1[:, h, 0:2, :],
                    start=False, stop=True, skip_group_check=True)
                nc.vector.tensor_copy(
                    osb[0:1, h, 0, 0 : 2 * HB], ops_[0:1, h, 0, 0 : 2 * HB])
                nc.sync.dma_start(oscr[s // 2 - 1 : s // 2, :, h, :],
                                  osb[0:1, h, 0, 0 : 2 * HB])

            # ---- end phase: transpose oscr [t, half, b2] -> out [b, t] ----
            nchunk = s // 128
            outsb = spool.tile([128, 2, nchunk, 128], F32, name="outsb")
            stg = [spool.tile([128, 2, HB], F32, name=f"stg{j}") for j in range(2)]
            for ci in range(nchunk):
                st = stg[ci % 2]
                nc.sync.dma_start(st[:], oscr[ci * 64 : (ci + 1) * 64, :, :, :])
                for g in range(2):
                    tp = gps[:, g, 0:128]
                    nc.tensor.transpose(tp, st[:, g, :], id_t[:])
                    nc.vector.tensor_copy(outsb[:, g, ci, :], tp)
            nc.sync.dma_start(outd[0:HB, :], outsb[:, 0, :, :])
            nc.sync.dma_start(outd[HB : 2 * HB, :], outsb[:, 1, :, :])

    if split_waits:
        _split_excess_waits(nc)
    return nc


_NC_CACHE = {}


def _get_nc(s=S, dbg=False):
    key = (s, dbg)
    if key not in _NC_CACHE:
        _NC_CACHE[key] = build_nc(s, dbg)
    return _NC_CACHE[key]


def run(x, weights, s=S, dbg=False, trace=False):
    """x: [s, B] fp32 (already squeezed); weights: dict of reference arrays."""
    Wg, wout1, wout2 = build_weights(**weights)
    nc = _get_nc(s, dbg)
    ident = np.eye(128, dtype=np.float32)
    in_maps = []
    for c in range(NCORES):
        xs = np.ascontiguousarray(x[:, c * BL : (c + 1) * BL])
        in_maps.append(
            {"xpair": build_xpair(xs, s),
             "Wg": Wg, "wout1": wout1, "wout2": wout2, "ident": ident}
        )
    res = run_bass_kernel_spmd(nc, in_maps, list(range(NCORES)), trace=trace)
    out = np.concatenate([res.results[c]["out"] for c in range(NCORES)], axis=0)
    return out, res


def kernel(x, Wih_f0, Whh_f0, b_f0, Wih_f1, Whh_f1, b_f1,
           Wih_b0, Whh_b0, b_b0, Wih_b1, Whh_b1, b_b1, Wlin, blin, future):
    assert int(future) == 0, "kernel hardcodes future=0"
    x = np.asarray(x, np.float32)
    s, b, _ = x.shape
    assert (s, b) == (S, B)
    weights = dict(
        Wih_f0=np.asarray(Wih_f0, np.float32), Whh_f0=np.asarray(Whh_f0, np.float32),
        b_f0=np.asarray(b_f0, np.float32),
        Wih_f1=np.asarray(Wih_f1, np.float32), Whh_f1=np.asarray(Whh_f1, np.float32),
        b_f1=np.asarray(b_f1, np.float32),
        Wih_b0=np.asarray(Wih_b0, np.float32), Whh_b0=np.asarray(Whh_b0, np.float32),
        b_b0=np.asarray(b_b0, np.float32),
        Wih_b1=np.asarray(Wih_b1, np.float32), Whh_b1=np.asarray(Whh_b1, np.float32),
        b_b1=np.asarray(b_b1, np.float32),
        Wlin=np.asarray(Wlin, np.float32), blin=np.asarray(blin, np.float32),
    )
    out, _ = run(x[:, :, 0], weights, s=S)
    return out
